# revision 44
# baseline (speedup 1.0000x reference)
"""Trainium2 Bass kernel for Enhanced Physics Attention with Sonata.

Contract: kernel(**inputs) takes FULL unsharded numpy inputs (as produced by
setup_inputs()) and returns the FULL [B, N, C] output. Internally shards
across 8 NeuronCores: core c handles batch c//2, token half c%2, and sonata
half c%2 (cross-attention partials). Two pairwise AllReduces: pooled slice
tokens after phase A, cross-attention numerators/denominators in phase B.

Math (fp32 PSUM accumulation, bf16 operands on the hot paths; rel err ~6e-3):
  Host folds Wslice+temperature into Wxs (s = x @ Wxs), SCALE into Wq/Wck,
  and casts x/weights to bf16.
  Phase A (token-major, lag-6 produce/consume software pipeline over
    pair-batched 128-token tiles): e = exp(s) [logits bounded ~3, no max
    needed], z = grouped sum (DVE), eh = e/z (gpsimd); pooling
    slice_token = eh^T @ (fx | 1) — the appended ones column yields
    slice_norm for free; eh^T (PE transpose + DVE 2x copy) stays fully
    resident in SBUF as bf16. Sonata projections + cross K/V are emitted
    interleaved mid-phase to fill engine slack.
  Phase B (stage-interleaved across heads so the in-order engine queues
    overlap the chains): slice self-attention (replicated) + sonata
    cross-attention over this core's sonata half; ones-column gives the
    softmax denominator, the pair AllReduce sums numerator+denominator
    over the full sonata. Both heads of a pair project through Wo into one
    full-bank PSUM tile (odd head via col tile_position) so os packs in
    one DVE copy.
  Phase C: out = eh_T.T @ (out_slice @ Wo), written back as bf16.

Self-contained: hardcodes all shapes; does not read sibling files.
"""

import contextlib
import sys

try:
    import concourse  # noqa: F401
except ImportError:
    sys.path.insert(0, "/opt/trn_rl_repo")

import ml_dtypes
import numpy as np

import concourse.bass as bass
import concourse.tile as tile
from concourse import bacc, mybir
from concourse.bass_utils import run_bass_kernel_spmd
from concourse.masks import make_identity

F32 = mybir.dt.float32
BF = mybir.dt.bfloat16
NPBF = np.dtype(ml_dtypes.bfloat16)
AF = mybir.ActivationFunctionType

# Problem shapes
B, N, C = 4, 16384, 256
H, D, G = 8, 64, 64
M, SD = 1024, 768
SCALE = D**-0.5
HG = H * G  # 512
HD = H * D  # 512
MH = M // 2  # sonata tokens per core (m-split across the pair)


def _bcast_ap(dram_ap: bass.AP, parts: int, n: int, offset: int = 0) -> bass.AP:
    """AP reading n contiguous DRAM floats, replicated across `parts` partitions."""
    return bass.AP(
        tensor=dram_ap.tensor,
        offset=dram_ap.offset + offset,
        ap=[[0, parts], [1, n]],
    )


def _fbcast(ap2d: bass.AP, rep: int) -> bass.AP:
    """[p, k] AP -> [p, k, rep] with step-0 innermost broadcast."""
    return bass.AP(tensor=ap2d.tensor, offset=ap2d.offset,
                   ap=[ap2d.ap[0], ap2d.ap[1], [0, rep]])


def _build(n_cores: int, T: int, flags: dict, no_collective: bool = False):
    """Build the per-core Bass module. T = tokens per core (multiple of 1024)."""
    assert T % 1024 == 0
    NSUP = T // 1024  # super-tiles (x loads)
    NT = T // 128  # 128-token tiles

    nc = bacc.Bacc(
        "TRN2", target_bir_lowering=False, debug=False, num_devices=n_cores
    )

    # ---- DRAM I/O ----
    xt = nc.dram_tensor("xt", [C, T], BF, kind="ExternalInput")  # x slice, transposed
    snt = nc.dram_tensor("snt", [SD, MH], BF, kind="ExternalInput")  # sonata half, T
    wxs = nc.dram_tensor("wxs", [C, HG], BF, kind="ExternalInput")
    wfx = nc.dram_tensor("wfx", [C, HD], BF, kind="ExternalInput")
    wsp = nc.dram_tensor("wsp", [SD, HD], BF, kind="ExternalInput")
    w5 = nc.dram_tensor("w5", [D, 5, D], BF, kind="ExternalInput")  # q,k,v,ck,cv
    wo = nc.dram_tensor("wo", [HD, C], BF, kind="ExternalInput")
    bqv = nc.dram_tensor("bqv", [3, D], F32, kind="ExternalInput")  # bq,bk,bv rows
    bck = nc.dram_tensor("bck", [D], F32, kind="ExternalInput")
    bsp = nc.dram_tensor("bsp", [HD], F32, kind="ExternalInput")
    if flags["bias_s"]:
        bias_s = nc.dram_tensor("bias_s", [HG], F32, kind="ExternalInput")
    if flags["bfx"]:
        bfx = nc.dram_tensor("bfx", [HD], F32, kind="ExternalInput")
    if flags["bcv"]:
        bcv = nc.dram_tensor("bcv", [D], F32, kind="ExternalInput")
    if flags["bo"]:
        bo = nc.dram_tensor("bo", [C], F32, kind="ExternalInput")
    y = nc.dram_tensor("y", [T, C], BF, kind="ExternalOutput")

    groups = [[2 * i, 2 * i + 1] for i in range(n_cores // 2)]

    def all_reduce(ar_in, ar_out):
        if no_collective:
            nc.gpsimd.dma_start(out=ar_out[:], in_=ar_in[:])
        else:
            nc.gpsimd.collective_compute(
                "AllReduce", mybir.AluOpType.add, replica_groups=groups,
                ins=[ar_in.opt()], outs=[ar_out.opt()])

    with tile.TileContext(nc) as tc, contextlib.ExitStack() as top:
        singles = top.enter_context(tc.tile_pool(name="singles", bufs=1))
        dram = top.enter_context(tc.tile_pool(name="dram", bufs=1, space="DRAM"))

        # ---- resident weights / inputs ----
        # Emission order matters: the in-order DMA queues must deliver wxs/wfx
        # and the first x super-tile before anything else so PE starts ASAP.
        wxs_sb = singles.tile([128, 2, HG], BF)
        wfx_sb = singles.tile([128, 2, HD], BF)
        wxs_re = wxs.ap().rearrange("(k p) n -> p k n", p=128)
        wfx_re = wfx.ap().rearrange("(k p) n -> p k n", p=128)
        # k0 chunks first so the very first matmuls can start sooner
        nc.sync.dma_start(out=wxs_sb[:, 0, :], in_=wxs_re[:, 0, :])
        ident = singles.tile([128, 128], BF)
        warm = singles.tile([1, 1], F32)
        nc.vector.memset(warm[:], 0.0)
        nc.scalar.activation(out=warm[:], in_=warm[:], func=AF.Exp)
        make_identity(nc, ident[:])
        # stacked eye(64)s: lets transposes consume partition-64-based [64,64]
        # sources directly (identity operand must share the source's base)
        ident2 = singles.tile([128, 64], BF)
        make_identity(nc, ident2[0:64, :])
        make_identity(nc, ident2[64:128, :])

        # tiles for deferred loads (DMAs emitted after the first super-tile)
        snt_sb = singles.tile([128, 6, MH], BF)
        wsp_sb = singles.tile([128, 6, HD], BF)
        w64 = singles.tile([64, 5, D], BF)  # wq,wk,wv,wck,wcv
        wo_sb = singles.tile([64, H, C], BF)

        def load_phaseB_weights():
            nc.sync.dma_start(out=snt_sb[:],
                              in_=snt.ap().rearrange("(k p) m -> p k m", p=128))
            nc.sync.dma_start(out=wsp_sb[:],
                              in_=wsp.ap().rearrange("(k p) n -> p k n", p=128))
            nc.sync.dma_start(out=w64[:], in_=w5.ap())
            nc.sync.dma_start(out=wo_sb[:],
                              in_=wo.ap().rearrange("(h d) c -> d h c", d=64))

        # eh^T: chunk c holds heads 2c,2c+1 stacked on partitions; fully
        # resident in SBUF as bf16 (4 * T * 2 bytes per partition).
        eT = singles.tile([128, 4, NT * 128], BF)
        # OS (slice-token outputs @ Wo), chunk-packed like eT
        os_sb = singles.tile([128, 4, C], BF)

        # (fx | 1) pooling rhs: manual 3-buffer rotation so the ones column
        # is preset exactly once per buffer.
        fxe_bufs = [singles.tile([128, 2, H, D + 1], BF, name=f"fxe{k}")
                    for k in range(4)]
        for fb in fxe_bufs:
            nc.vector.memset(fb[:, :, :, D], 1.0)

        # small per-partition bias columns
        bqv_c = singles.tile([64, 3], F32)
        nc.sync.dma_start(out=bqv_c[:], in_=bqv.ap().rearrange("q d -> d q"))
        bck_c = singles.tile([64, 1], F32)
        nc.sync.dma_start(out=bck_c[:], in_=bck.ap().rearrange("(d o) -> d o", o=1))
        bsp_c = singles.tile([128, 4], F32)
        nc.sync.dma_start(out=bsp_c[:], in_=bsp.ap().rearrange("(k p) -> p k", p=128))

        if flags["bias_s"]:
            bias_s_bc = singles.tile([128, HG], F32)
            nc.sync.dma_start(out=bias_s_bc[:], in_=_bcast_ap(bias_s.ap(), 128, HG))
        if flags["bcv"]:
            bcv_bc = singles.tile([128, D], F32)
            nc.sync.dma_start(out=bcv_bc[:], in_=_bcast_ap(bcv.ap(), 128, D))
        if flags["bo"]:
            # bo/H replicated on all 128 partitions (see phase B T4: each of
            # the H unpool rows carries bo/H, and per-head eh sums to 1)
            bo_bc = singles.tile([128, C], F32)
            nc.sync.dma_start(out=bo_bc[:], in_=_bcast_ap(bo.ap(), 128, C))
            nc.vector.tensor_scalar_mul(out=bo_bc[:], in0=bo_bc[:],
                                        scalar1=1.0 / H)
        if flags["bfx"]:
            bfx_bc = singles.tile([128, D], F32)

        # sonata-side SBUF tiles (produced during phase A, consumed in phase B)
        sfT = singles.tile([64, H, MH], BF)  # sf^T [d, h, m] head-major
        ksT_all = singles.tile([64, H, MH], BF)
        NMC = MH // 128  # m-chunks on this core
        vse_all = singles.tile([128, H, NMC, D + 1], BF)
        nc.vector.memset(vse_all[:, :, :, D : D + 1], 1.0)

        # ---------------- Phase A ----------------
        LAG = 6  # tiles of software-pipeline lag (even: produce/consume pair tiles)
        with contextlib.ExitStack() as phA:
            pA = phA.enter_context(tc.tile_pool(name="pA", bufs=5))
            pAx = phA.enter_context(tc.tile_pool(name="pAx", bufs=8))
            psA = phA.enter_context(tc.tile_pool(name="psA", bufs=2, space="PSUM"))
            psAcc = phA.enter_context(tc.tile_pool(name="psAcc", bufs=1, space="PSUM"))
            psH = phA.enter_context(tc.tile_pool(name="psH", bufs=1, space="PSUM"))

            # All 8 head accumulators live on partitions 0-63 across two full
            # PSUM banks (2KB-exact per-partition stride). Only the first
            # matmul touching each bank (h==0 / h==4 at tile 0) uses
            # start=True: the bank-wide has_written clear makes heads 1-3 /
            # 5-7 of tile 0 overwrite, and all later tiles accumulate.
            pool_acc = psAcc.tile([64, H, 128], F32, name="poolacc", tag="acc")

            # --- hoisted phase-B producers: sonata projection + cross K/V ---
            # Emitted interleaved into phase A to fill engine slack.
            def unit_sf(hd):  # hd chunk = heads 2hd, 2hd+1
                ps = psH.tile([128, MH], F32, tag="hps")
                for sd in range(6):
                    nc.tensor.matmul(
                        ps[:], lhsT=wsp_sb[:, sd, hd * 128 : (hd + 1) * 128],
                        rhs=snt_sb[:, sd, :], start=(sd == 0), stop=(sd == 5))
                up = pA.tile([128, MH], BF, tag="sfup")
                if flags["bsp"]:
                    nc.scalar.activation(out=sfT[:, 2 * hd, :], in_=ps[0:64, :],
                                         func=AF.Identity,
                                         bias=bsp_c[0:64, hd : hd + 1])
                    nc.scalar.activation(out=up[64:128, :], in_=ps[64:128, :],
                                         func=AF.Identity,
                                         bias=bsp_c[64:128, hd : hd + 1])
                else:
                    nc.vector.tensor_copy(out=sfT[:, 2 * hd, :], in_=ps[0:64, :])
                    nc.vector.tensor_copy(out=up[64:128, :], in_=ps[64:128, :])
                nc.gpsimd.dma_start(out=sfT[:, 2 * hd + 1, :], in_=up[64:128, :])

            def unit_kv(h):
                kp = psH.tile([64, MH], F32, tag="hps")
                nc.tensor.matmul(kp[:], lhsT=w64[:, 3, :], rhs=sfT[:, h, :],
                                 start=True, stop=True)
                nc.scalar.activation(out=ksT_all[:, h, :], in_=kp[:],
                                     func=AF.Identity, bias=bck_c[:])
                vp4 = psH.tile([128, NMC, 128], F32, tag="hps")
                for mo in range(NMC):
                    nc.tensor.matmul(vp4[:, mo, 0:64],
                                     lhsT=sfT[:, h, mo * 128 : (mo + 1) * 128],
                                     rhs=w64[:, 4, :], start=True, stop=True)
                if flags["bcv"]:
                    bcv4 = bass.AP(tensor=bcv_bc[:].tensor, offset=bcv_bc[:].offset,
                                   ap=[bcv_bc[:].ap[0], [0, NMC], [1, D]])
                    nc.vector.tensor_add(out=vse_all[:, h, :, 0:D],
                                         in0=vp4[:, :, 0:64], in1=bcv4)
                else:
                    nc.vector.tensor_copy(out=vse_all[:, h, :, 0:D],
                                          in_=vp4[:, :, 0:64])

            units = [lambda hd=hd: unit_sf(hd) for hd in range(4)]
            units += [lambda h=h: unit_kv(h) for h in range(H)]
            # spread units over mid-phase tiles (needs weights from si==0 DMAs)
            if NT >= 52:
                unit_at = {16 + (3 * k) // 2 * 2: k for k in range(len(units))}
            else:
                unit_at = {}
            units_done = [False] * len(units)

            handles = {}

            def produce2(i0, xt_sb, j0):
                # tiles i0, i0+1 batched: one ACT/DVE/Pool op per stage pair
                # amortizes the fixed memory-access cost of each instruction.
                s_ps = psA.tile([128, 2, HG], F32, tag="s", bufs=1)
                fx_ps = psA.tile([128, 2, HD], F32, tag="fx", bufs=1)
                for t in range(2):
                    tok = slice((j0 + t) * 128, (j0 + t + 1) * 128)
                    nc.tensor.matmul(s_ps[:, t, :], lhsT=xt_sb[:, 0, tok],
                                     rhs=wxs_sb[:, 0, :], start=True, stop=False)
                    nc.tensor.matmul(s_ps[:, t, :], lhsT=xt_sb[:, 1, tok],
                                     rhs=wxs_sb[:, 1, :], start=False, stop=True)
                for t in range(2):
                    tok = slice((j0 + t) * 128, (j0 + t + 1) * 128)
                    nc.tensor.matmul(fx_ps[:, t, :], lhsT=xt_sb[:, 0, tok],
                                     rhs=wfx_sb[:, 0, :], start=True, stop=False)
                    nc.tensor.matmul(fx_ps[:, t, :], lhsT=xt_sb[:, 1, tok],
                                     rhs=wfx_sb[:, 1, :], start=False, stop=True)

                e2 = pA.tile([128, 2, H, G], BF, tag="e")
                ef = e2.rearrange("p t a b -> p (t a b)")
                if flags["bias_s"]:
                    bias2 = bass.AP(tensor=bias_s_bc[:].tensor,
                                    offset=bias_s_bc[:].offset,
                                    ap=[bias_s_bc[:].ap[0], [0, 2], [1, HG]])
                    s_sb = pA.tile([128, 2, HG], F32, tag="ssb")
                    nc.vector.tensor_add(out=s_sb[:], in0=s_ps[:], in1=bias2)
                    nc.scalar.activation(out=ef, in_=s_sb.rearrange(
                        "p t n -> p (t n)"), func=AF.Exp)
                else:
                    nc.scalar.activation(
                        out=ef, in_=s_ps.rearrange("p t n -> p (t n)"),
                        func=AF.Exp)

                z2 = pA.tile([128, 2, H], F32, tag="z")
                nc.vector.reduce_sum(out=z2[:], in_=e2[:],
                                     axis=mybir.AxisListType.X)
                nc.vector.reciprocal(out=z2.rearrange("p t a -> p (t a)"),
                                     in_=z2.rearrange("p t a -> p (t a)"))
                z2ap = z2[:]
                zrb = bass.AP(tensor=z2ap.tensor, offset=z2ap.offset,
                              ap=[z2ap.ap[0], [H, 2], [1, H], [0, G]])

                eh2 = pA.tile([128, 2, H, G], BF, tag="eh")
                nc.gpsimd.tensor_tensor(out=eh2[:], in0=e2[:], in1=zrb,
                                        op=mybir.AluOpType.mult)
                # pooling rhs = (fx | 1), ones columns preset per buffer
                fxe2 = fxe_bufs[(i0 // 2) % len(fxe_bufs)]
                nc.scalar.activation(
                    out=fxe2[:, :, :, 0:D],
                    in_=fx_ps.rearrange("p t (a b) -> p t a b", a=H),
                    func=AF.Copy)
                handles[i0] = (eh2, fxe2)

            def consume2(i0):
                eh2, fxe2 = handles.pop(i0)
                for t in range(2):
                    i = i0 + t
                    for h in range(H):
                        nc.tensor.matmul(
                            pool_acc[0:64, h, 0 : D + 1],
                            lhsT=eh2[:, t, h, :], rhs=fxe2[:, t, h, :],
                            start=(i == 0 and h % 4 == 0), stop=(i == NT - 1),
                            skip_group_check=True)

                etp = psA.tile([128, 2, 4, 128], BF, tag="etp", bufs=1)
                ehf = eh2.rearrange("p t a b -> p t (a b)")
                for t in range(2):
                    for cc in range(4):
                        nc.tensor.transpose(etp[:, t, cc, :],
                                            ehf[:, t, cc * 128 : (cc + 1) * 128],
                                            ident[:])
                nc.vector.tensor_copy(
                    out=eT[:, :, i0 * 128 : (i0 + 2) * 128].rearrange(
                        "p c (t k) -> p c t k", t=2),
                    in_=etp.rearrange("p t c k -> p c t k"))

            xt_re = xt.ap().rearrange("(k p) n -> p k n", p=128)
            # Prefetch every x super-tile up front: the SP queue then serves
            # the eh^T transposes without ever blocking an x load behind them.
            xt_tiles = []
            for si in range(NSUP):
                xt_sb = pAx.tile([128, 2, 1024], BF, tag="xt", name=f"xt{si}")
                sl = slice(si * 1024, (si + 1) * 1024)
                if si == 0:
                    # finest-grained first loads: the tile-0 matmuls only need
                    # the first 256 tokens of each k-chunk
                    nc.sync.dma_start(out=xt_sb[:, 0, 0:256], in_=xt_re[:, 0, 0:256])
                    nc.sync.dma_start(out=wfx_sb[:, 0, :], in_=wfx_re[:, 0, :])
                    nc.sync.dma_start(out=xt_sb[:, 0, 256:1024],
                                      in_=xt_re[:, 0, slice(256, 1024)])
                    nc.sync.dma_start(out=wxs_sb[:, 1, :], in_=wxs_re[:, 1, :])
                    nc.sync.dma_start(out=xt_sb[:, 1, 0:256], in_=xt_re[:, 1, 0:256])
                    nc.sync.dma_start(out=wfx_sb[:, 1, :], in_=wfx_re[:, 1, :])
                    nc.sync.dma_start(out=xt_sb[:, 1, 256:1024],
                                      in_=xt_re[:, 1, slice(256, 1024)])
                else:
                    nc.sync.dma_start(out=xt_sb[:], in_=xt_re[:, :, sl])
                xt_tiles.append(xt_sb)
            for si in range(NSUP):
                xt_sb = xt_tiles[si]
                for j in range(0, 8, 2):
                    i = si * 8 + j
                    produce2(i, xt_sb, j)
                    if i >= LAG:
                        consume2(i - LAG)
                    if i in unit_at:
                        k = unit_at.pop(i)
                        units[k]()
                        units_done[k] = True
                if si == 0:
                    load_phaseB_weights()
            for i in range(NT - LAG, NT, 2):
                consume2(i)
            for k, u in enumerate(units):  # emit any units not yet scheduled
                if not units_done[k]:
                    u()

            # pooled partials -> AllReduce across the pair. The accumulator is
            # [64p, 8h, 65]; the AR payload (and phase B) use the pair layout
            # [128p = 64p x 2(h odd/even), 4 pairs, 65] — the DRAM staging DMA
            # applies the permutation (DRAM side is fully linear).
            pool_sb = pA.tile([64, H, D + 1], F32, tag="poolsb")
            nc.scalar.activation(out=pool_sb[:], in_=pool_acc[:, :, 0 : D + 1],
                                 func=AF.Copy)
            ar_in = dram.tile([128, 4 * (D + 1)], F32)
            ar_out = dram.tile([128, 4 * (D + 1)], F32)
            ar_in_ap = ar_in[:]
            ar_in_perm = bass.AP(
                tensor=ar_in_ap.tensor, offset=ar_in_ap.offset,
                # lockstep with pool_sb [64p][4 hp][2 hh][65]: dram row
                # p + 64*hh, column hp*65 + b
                ap=[[4 * (D + 1), 64], [D + 1, 4], [64 * 4 * (D + 1), 2], [1, D + 1]])
            nc.sync.dma_start(
                out=ar_in_perm,
                in_=pool_sb.rearrange("p (a c) b -> p a c b", a=4))
            all_reduce(ar_in, ar_out)

        # ---------------- Phase B ----------------
        with contextlib.ExitStack() as phB:
            pBw = phB.enter_context(tc.tile_pool(name="pBw", bufs=1))
            pB = phB.enter_context(tc.tile_pool(name="pB", bufs=2))
            pBh = phB.enter_context(tc.tile_pool(name="pBh", bufs=8))
            psB = phB.enter_context(tc.tile_pool(name="psB", bufs=2, space="PSUM"))
            psBs = phB.enter_context(tc.tile_pool(name="psBs", bufs=1, space="PSUM"))

            pool_red = pB.tile([128, 4, D + 1], F32, tag="poolred")
            nc.sync.dma_start(out=pool_red.rearrange("p a b -> p (a b)"), in_=ar_out[:])

            # Stage-interleaved emission: each stage is emitted for all heads
            # before the next stage, so the in-order engine queues overlap the
            # independent per-head chains instead of running them serially.
            ocst = pBw.tile([64, H, D + 1], F32)  # cross-attn partials, h-major
            osT_all = pBw.tile([64, H, 64], BF)  # self-attn out^T per head
            heads = [(hp, hh) for hp in range(4) for hh in range(2)]
            st2s, stTs, qkvTs = {}, {}, {}
            d_a, d_ea, d_za, d_pa, d_vsb, d_pat, d_ecT, d_oc = ({} for _ in range(8))

            for hp in range(4):  # S1: slice-token normalize
                pr = pool_red[:, hp, :]  # [128, 65]: heads 2hp (low), 2hp+1 (hi)
                nrm = pBh.tile([128, 1], F32, tag="nrm")
                nc.vector.tensor_scalar_add(out=nrm[:], in0=pr[:, D : D + 1],
                                            scalar1=1e-5)
                nc.vector.reciprocal(out=nrm[:], in_=nrm[:])
                st2 = pBh.tile([128, D], BF, tag="st2")
                if flags["bfx"]:
                    for hh in range(2):
                        h = 2 * hp + hh
                        sl = slice(hh * 64, hh * 64 + 64)
                        nc.sync.dma_start(out=bfx_bc[sl, :],
                                          in_=_bcast_ap(bfx.ap(), 64, D, offset=h * D))
                    tmpb = pBh.tile([128, D], F32, tag="tmpb")
                    nc.vector.tensor_scalar_mul(out=tmpb[:], in0=bfx_bc[:],
                                                scalar1=pr[:, D : D + 1])
                    nc.vector.tensor_add(out=tmpb[:], in0=tmpb[:], in1=pr[:, 0:D])
                    nc.vector.tensor_scalar_mul(out=st2[:], in0=tmpb[:], scalar1=nrm[:])
                else:
                    nc.vector.tensor_scalar_mul(out=st2[:], in0=pr[:, 0:D],
                                                scalar1=nrm[:])
                st2s[hp] = st2
            for hp in range(4):  # S2: transpose slice tokens
                stT_ps = psBs.tile([64, 128], BF, tag="small", bufs=4)
                nc.tensor.transpose(stT_ps[:], st2s[hp][:], ident[:])
                stT = pBh.tile([64, 128], BF, tag="stT")
                nc.vector.tensor_copy(out=stT[:], in_=stT_ps[:])
                stTs[hp] = stT
            for hp in range(4):  # S3: q/k/v projections (batched per pair)
                qkvTs[hp] = pBh.tile([64, 3, 128], BF, tag="qkvT",
                                     name=f"qkvT{hp}")
                qp = psBs.tile([64, 3, 128], F32, tag="small", bufs=4,
                               name=f"qp{hp}")
                for idx in range(3):
                    nc.tensor.matmul(qp[:, idx, :], lhsT=w64[:, idx, :],
                                     rhs=stTs[hp][:], start=True, stop=True)
                if flags["bqv"]:
                    for idx in range(3):
                        nc.scalar.activation(out=qkvTs[hp][:, idx, :],
                                             in_=qp[:, idx, :], func=AF.Identity,
                                             bias=bqv_c[:, idx : idx + 1])
                elif hp % 2 == 0:
                    nc.vector.tensor_copy(out=qkvTs[hp][:], in_=qp[:])
                else:
                    nc.scalar.activation(out=qkvTs[hp].rearrange("p a b -> p (a b)"),
                                         in_=qp.rearrange("p a b -> p (a b)"),
                                         func=AF.Copy)
            for hp, hh in heads:  # S4: self-attention logits
                hs = slice(hh * 64, hh * 64 + 64)
                a_ps = psBs.tile([64, 64], F32, tag="small", bufs=4)
                nc.tensor.matmul(a_ps[:], lhsT=qkvTs[hp][:, 0, hs],
                                 rhs=qkvTs[hp][:, 1, hs], start=True, stop=True)
                d_a[(hp, hh)] = a_ps
            for hp, hh in heads:  # S5: softmax exp, then row sums on DVE
                ea = pBh.tile([64, 64], F32, tag="ea")
                nc.scalar.activation(out=ea[:], in_=d_a.pop((hp, hh))[:],
                                     func=AF.Exp)
                za = pBh.tile([64, 1], F32, tag="za")
                nc.vector.reduce_sum(out=za[:], in_=ea[:],
                                     axis=mybir.AxisListType.X)
                d_ea[(hp, hh)], d_za[(hp, hh)] = ea, za
            for hp, hh in heads:  # S6: normalize attention
                za = d_za.pop((hp, hh))
                nc.vector.reciprocal(out=za[:], in_=za[:])
                pa = pBh.tile([64, 64], BF, tag="pa")
                nc.vector.tensor_scalar_mul(out=pa[:], in0=d_ea.pop((hp, hh))[:],
                                            scalar1=za[:])
                d_pa[(hp, hh)] = pa
            for hp, hh in heads:  # S7: transpose v and attention (batched)
                hs = slice(hh * 64, hh * 64 + 64)
                vp_ps = psBs.tile([64, 2, 64], BF, tag="small", bufs=4)
                nc.tensor.transpose(vp_ps[:, 0, :], qkvTs[hp][:, 2, hs],
                                    ident[0:64, 0:64])
                nc.tensor.transpose(vp_ps[:, 1, :], d_pa.pop((hp, hh))[:],
                                    ident[0:64, 0:64])
                vpat = pBh.tile([64, 2, 64], BF, tag="vpat")
                nc.vector.tensor_copy(out=vpat[:], in_=vp_ps[:])
                d_vsb[(hp, hh)] = vpat
            for hp, hh in heads:  # S8: self-attention output
                h = 2 * hp + hh
                vpat = d_vsb.pop((hp, hh))
                osf_ps = psBs.tile([64, 64], F32, tag="small", bufs=4)
                nc.tensor.matmul(osf_ps[:], lhsT=vpat[:, 0, :],
                                 rhs=vpat[:, 1, :], start=True, stop=True)
                nc.vector.tensor_copy(out=osT_all[:, h, :], in_=osf_ps[:])
            for hp, hh in heads:  # S9: cross-attention logits
                h = 2 * hp + hh
                ct_ps = psB.tile([128, NMC, 64], F32, tag="ct", bufs=2)
                for mo in range(NMC):
                    nc.tensor.matmul(ct_ps[:, mo, :],
                                     lhsT=ksT_all[:, h, mo * 128 : (mo + 1) * 128],
                                     rhs=osT_all[:, h, :], start=True, stop=True)
                d_a[(hp, hh)] = ct_ps
            for hp, hh in heads:  # S10: cross-attention exp
                ecT = pBh.tile([128, NMC, 64], BF, tag="ecT")
                ct_ps = d_a.pop((hp, hh))
                nc.scalar.activation(out=ecT.rearrange("p a b -> p (a b)"),
                                     in_=ct_ps.rearrange("p a b -> p (a b)"),
                                     func=AF.Exp)
                d_ecT[(hp, hh)] = ecT
            for hp, hh in heads:  # S11: cross numerator/denominator partials
                h = 2 * hp + hh
                ecT = d_ecT.pop((hp, hh))
                oc_ps = psBs.tile([64, 128], F32, tag="small", bufs=4)
                for mo in range(NMC):
                    nc.tensor.matmul(oc_ps[:, 0 : D + 1], lhsT=ecT[:, mo, :],
                                     rhs=vse_all[:, h, mo, :],
                                     start=(mo == 0), stop=(mo == NMC - 1))
                d_oc[(hp, hh)] = oc_ps
            for hp, hh in heads:  # S12: pack for the pair AllReduce
                h = 2 * hp + hh
                nc.vector.tensor_copy(out=ocst[0:64, h, :],
                                      in_=d_oc.pop((hp, hh))[:, 0 : D + 1])

            ar2_in = dram.tile([128, 4 * (D + 1)], F32)
            ar2_out = dram.tile([128, 4 * (D + 1)], F32)
            ar2_ap = ar2_in[:]
            ar2_perm = bass.AP(
                tensor=ar2_ap.tensor, offset=ar2_ap.offset,
                ap=[[4 * (D + 1), 64], [D + 1, 4], [64 * 4 * (D + 1), 2], [1, D + 1]])
            nc.sync.dma_start(out=ar2_perm,
                               in_=ocst.rearrange("p (a c) b -> p a c b", a=4))
            all_reduce(ar2_in, ar2_out)
            ocred = pB.tile([128, 4, D + 1], F32, tag="ocred")
            nc.sync.dma_start(out=ocred.rearrange("p a b -> p (a b)"), in_=ar2_out[:])

            # finish cross-attention + OS, stage-interleaved across heads
            oc2s, osfTs = {}, {}
            for hp in range(4):  # T1: cross-softmax normalize
                oc2 = pBh.tile([128, D], BF, tag="oc2")
                zc = pBh.tile([128, 1], F32, tag="zc")
                nc.vector.reciprocal(out=zc[:], in_=ocred[:, hp, D : D + 1])
                nc.vector.tensor_scalar_mul(out=oc2[:], in0=ocred[:, hp, 0:D],
                                            scalar1=zc[:])
                oc2s[hp] = oc2
            for hp, hh in heads:  # T2: transpose + residual add
                h = 2 * hp + hh
                src = oc2s[hp][0:64, :] if hh == 0 else oc2s[hp][64:128, :]
                idn = ident2[0:64, :] if hh == 0 else ident2[64:128, :]
                ocT_ps = psBs.tile([64, 64], BF, tag="small", bufs=4)
                nc.tensor.transpose(ocT_ps[:], src, idn)
                osfT = pBh.tile([64, 64], BF, tag="osfT")
                nc.vector.tensor_add(out=osfT[:], in0=ocT_ps[:],
                                     in1=osT_all[:, h, :])
                osfTs[(hp, hh)] = osfT
            osps = {}
            for hp, hh in heads:  # T3: project through Wo (pair shares a bank)
                h = 2 * hp + hh
                if hh == 0:
                    osps[hp] = psBs.tile([128, 512], F32, tag="osp", bufs=2,
                                         name=f"osp{hp}")
                nc.tensor.matmul(osps[hp][hh * 64 : hh * 64 + 64, 0:C],
                                 lhsT=osfTs.pop((hp, hh))[:],
                                 rhs=wo_sb[:, h, :], start=True, stop=True)
            for hp in range(4):  # T4: pack os_sb in one copy per pair
                osr = osps.pop(hp)
                if flags["bo"]:
                    nc.vector.tensor_add(out=os_sb[:, hp, :], in0=osr[:, 0:C],
                                         in1=bo_bc[:])
                else:
                    nc.vector.tensor_copy(out=os_sb[:, hp, :], in_=osr[:, 0:C])

        # ---------------- Phase C ----------------
        with contextlib.ExitStack() as phC:
            pC = phC.enter_context(tc.tile_pool(name="pC", bufs=2))
            psC = phC.enter_context(tc.tile_pool(name="psC", bufs=6, space="PSUM"))
            y_re = y.ap().rearrange("(s j p) c -> s p j c", j=8, p=128)
            for si in range(NSUP):
                stg = pC.tile([128, 8, C], BF, tag="stg")
                for j in range(8):
                    i = si * 8 + j
                    o_ps = psC.tile([128, C], F32, tag="o")
                    for cc in range(4):
                        nc.tensor.matmul(o_ps[:],
                                         lhsT=eT[:, cc, i * 128 : (i + 1) * 128],
                                         rhs=os_sb[:, cc, :],
                                         start=(cc == 0), stop=(cc == 3))
                    if i % 2 == 0:
                        nc.scalar.activation(out=stg[:, j, :], in_=o_ps[:],
                                             func=AF.Copy)
                    else:
                        nc.vector.tensor_copy(out=stg[:, j, :], in_=o_ps[:])
                nc.sync.dma_start(out=y_re[si], in_=stg[:])

    nc.compile()
    return nc


_CACHE: dict = {}


def _get_nc(n_cores: int, T: int, flags_key: tuple):
    key = (n_cores, T, flags_key)
    if key not in _CACHE:
        flags = dict(zip(("bias_s", "bqv", "bsp", "bck", "bfx", "bcv", "bo"), flags_key))
        _CACHE[key] = _build(n_cores, T, flags)
    return _CACHE[key]


def prep_inputs(inputs: dict, n_cores: int, T: int):
    """Host-side prep: transposes, weight folding, bf16 casts, per-core maps."""
    f32 = np.float32
    x = np.asarray(inputs["x"], f32)
    snt = np.asarray(inputs["sonata_features"], f32)
    temp = np.asarray(inputs["temperature"], f32).reshape(H)
    Wx, bx = np.asarray(inputs["Wx"], f32), np.asarray(inputs["bx"], f32)
    Wfx, bfx = np.asarray(inputs["Wfx"], f32), np.asarray(inputs["bfx"], f32)
    Wsl, bsl = np.asarray(inputs["Wslice"], f32), np.asarray(inputs["bslice"], f32)
    Wq, bq = np.asarray(inputs["Wq"], f32), np.asarray(inputs["bq"], f32)
    Wk, bk = np.asarray(inputs["Wk"], f32), np.asarray(inputs["bk"], f32)
    Wv, bv = np.asarray(inputs["Wv"], f32), np.asarray(inputs["bv"], f32)
    Wsp, bsp = np.asarray(inputs["Wsp"], f32), np.asarray(inputs["bsp"], f32)
    Wck, bck = np.asarray(inputs["Wck"], f32), np.asarray(inputs["bck"], f32)
    Wcv, bcv = np.asarray(inputs["Wcv"], f32), np.asarray(inputs["bcv"], f32)
    Wo, bo = np.asarray(inputs["Wo"], f32), np.asarray(inputs["bo"], f32)

    Wxs = np.zeros((C, HG), f32)
    bias_s = np.zeros((HG,), f32)
    for h in range(H):
        Wxs[:, h * G : (h + 1) * G] = (Wx[:, h * D : (h + 1) * D] @ Wsl) / temp[h]
        bias_s[h * G : (h + 1) * G] = (bx[h * D : (h + 1) * D] @ Wsl + bsl) / temp[h]
    flags = {
        "bias_s": bool(np.any(bias_s != 0)),
        "bqv": bool(np.any(bq != 0) or np.any(bk != 0) or np.any(bv != 0)),
        "bsp": bool(np.any(bsp != 0)),
        "bck": bool(np.any(bck != 0)),
        "bfx": bool(np.any(bfx != 0)),
        "bcv": bool(np.any(bcv != 0)),
        "bo": bool(np.any(bo != 0)),
    }
    w5 = np.stack([Wq * SCALE, Wk, Wv, Wck * SCALE, Wcv], axis=1)  # [D, 5, D]
    shared = {
        "wxs": np.ascontiguousarray(Wxs).astype(NPBF),
        "wfx": np.ascontiguousarray(Wfx).astype(NPBF),
        "wsp": np.ascontiguousarray(Wsp).astype(NPBF),
        "w5": np.ascontiguousarray(w5).astype(NPBF),
        "wo": np.ascontiguousarray(Wo).astype(NPBF),
        "bqv": np.ascontiguousarray(np.stack([bq * SCALE, bk, bv])),
        "bck": np.ascontiguousarray(bck * SCALE),
        "bsp": np.ascontiguousarray(bsp),
    }
    if flags["bias_s"]:
        shared["bias_s"] = bias_s
    if flags["bfx"]:
        shared["bfx"] = bfx
    if flags["bcv"]:
        shared["bcv"] = bcv
    if flags["bo"]:
        shared["bo"] = bo

    in_maps = []
    for c in range(n_cores):
        b, half = c // 2, c % 2
        xt_c = np.ascontiguousarray(x[b, half * T : (half + 1) * T, :].T).astype(NPBF)
        snt_c = np.ascontiguousarray(
            snt[b].T[:, half * MH : (half + 1) * MH]).astype(NPBF)
        in_maps.append({"xt": xt_c, "snt": snt_c, **shared})
    return in_maps, flags


def run(inputs: dict, n_cores: int = 8, T: int = N // 2, **spmd_kwargs):
    in_maps, flags = prep_inputs(inputs, n_cores, T)
    nc = _get_nc(n_cores, T, tuple(flags.values()))
    res = run_bass_kernel_spmd(nc, in_maps, core_ids=list(range(n_cores)),
                               **spmd_kwargs)
    out = np.zeros((B, N, C), np.float32)
    for c in range(n_cores):
        b, half = c // 2, c % 2
        out[b, half * T : (half + 1) * T, :] = np.asarray(
            res.results[c]["y"]).astype(np.float32)
    return out, res


def kernel(**inputs) -> np.ndarray:
    out, _ = run(inputs)
    return out


# revision 51
# speedup vs baseline: 1.0070x; 1.0070x over previous
"""Trainium2 Bass kernel for Enhanced Physics Attention with Sonata.

Contract: kernel(**inputs) takes FULL unsharded numpy inputs (as produced by
setup_inputs()) and returns the FULL [B, N, C] output. Internally shards
across 8 NeuronCores: core c handles batch c//2, token half c%2, and sonata
half c%2 (cross-attention partials). Two pairwise AllReduces: pooled slice
tokens after phase A, cross-attention numerators/denominators in phase B.

Math (fp32 PSUM accumulation, bf16 operands on the hot paths; rel err ~6e-3):
  Host folds Wslice+temperature into Wxs (s = x @ Wxs), SCALE into Wq/Wck,
  and casts x/weights to bf16.
  Phase A (token-major, lag-6 produce/consume software pipeline over
    pair-batched 128-token tiles): e = exp(s) [logits bounded ~3, no max
    needed], z = grouped sum (DVE), eh = e/z (gpsimd); pooling
    slice_token = eh^T @ (fx | 1) — the appended ones column yields
    slice_norm for free; eh^T (PE transpose + DVE 2x copy) stays fully
    resident in SBUF as bf16. Sonata projections + cross K/V are emitted
    interleaved mid-phase to fill engine slack.
  Phase B (stage-interleaved across heads so the in-order engine queues
    overlap the chains): slice self-attention (replicated) + sonata
    cross-attention over this core's sonata half; ones-column gives the
    softmax denominator, the pair AllReduce sums numerator+denominator
    over the full sonata. Both heads of a pair project through Wo into one
    full-bank PSUM tile (odd head via col tile_position) so os packs in
    one DVE copy.
  Phase C: out = eh_T.T @ (out_slice @ Wo), written back as bf16.

Self-contained: hardcodes all shapes; does not read sibling files.
"""

import contextlib
import sys

try:
    import concourse  # noqa: F401
except ImportError:
    sys.path.insert(0, "/opt/trn_rl_repo")

import ml_dtypes
import numpy as np

import concourse.bass as bass
import concourse.tile as tile
from concourse import bacc, mybir
from concourse.bass_utils import run_bass_kernel_spmd
from concourse.masks import make_identity

F32 = mybir.dt.float32
BF = mybir.dt.bfloat16
NPBF = np.dtype(ml_dtypes.bfloat16)
AF = mybir.ActivationFunctionType

# Problem shapes
B, N, C = 4, 16384, 256
H, D, G = 8, 64, 64
M, SD = 1024, 768
SCALE = D**-0.5
HG = H * G  # 512
HD = H * D  # 512
MH = M // 2  # sonata tokens per core (m-split across the pair)


def _bcast_ap(dram_ap: bass.AP, parts: int, n: int, offset: int = 0) -> bass.AP:
    """AP reading n contiguous DRAM floats, replicated across `parts` partitions."""
    return bass.AP(
        tensor=dram_ap.tensor,
        offset=dram_ap.offset + offset,
        ap=[[0, parts], [1, n]],
    )


def _fbcast(ap2d: bass.AP, rep: int) -> bass.AP:
    """[p, k] AP -> [p, k, rep] with step-0 innermost broadcast."""
    return bass.AP(tensor=ap2d.tensor, offset=ap2d.offset,
                   ap=[ap2d.ap[0], ap2d.ap[1], [0, rep]])


def _build(n_cores: int, T: int, flags: dict, no_collective: bool = False):
    """Build the per-core Bass module. T = tokens per core (multiple of 1024)."""
    assert T % 1024 == 0
    NSUP = T // 1024  # super-tiles (x loads)
    NT = T // 128  # 128-token tiles

    nc = bacc.Bacc(
        "TRN2", target_bir_lowering=False, debug=False, num_devices=n_cores
    )

    # ---- DRAM I/O ----
    xt = nc.dram_tensor("xt", [C, T], BF, kind="ExternalInput")  # x slice, transposed
    snt = nc.dram_tensor("snt", [SD, MH], BF, kind="ExternalInput")  # sonata half, T
    wxs = nc.dram_tensor("wxs", [C, HG], BF, kind="ExternalInput")
    wfx = nc.dram_tensor("wfx", [C, HD], BF, kind="ExternalInput")
    wsp = nc.dram_tensor("wsp", [SD, HD], BF, kind="ExternalInput")
    w5 = nc.dram_tensor("w5", [D, 5, D], BF, kind="ExternalInput")  # q,k,v,ck,cv
    wo = nc.dram_tensor("wo", [HD, C], BF, kind="ExternalInput")
    bqv = nc.dram_tensor("bqv", [3, D], F32, kind="ExternalInput")  # bq,bk,bv rows
    bck = nc.dram_tensor("bck", [D], F32, kind="ExternalInput")
    bsp = nc.dram_tensor("bsp", [HD], F32, kind="ExternalInput")
    if flags["bias_s"]:
        bias_s = nc.dram_tensor("bias_s", [HG], F32, kind="ExternalInput")
    if flags["bfx"]:
        bfx = nc.dram_tensor("bfx", [HD], F32, kind="ExternalInput")
    if flags["bcv"]:
        bcv = nc.dram_tensor("bcv", [D], F32, kind="ExternalInput")
    if flags["bo"]:
        bo = nc.dram_tensor("bo", [C], F32, kind="ExternalInput")
    y = nc.dram_tensor("y", [T, C], BF, kind="ExternalOutput")

    groups = [[2 * i, 2 * i + 1] for i in range(n_cores // 2)]

    def all_reduce(ar_in, ar_out):
        if no_collective:
            nc.gpsimd.dma_start(out=ar_out[:], in_=ar_in[:])
        else:
            nc.gpsimd.collective_compute(
                "AllReduce", mybir.AluOpType.add, replica_groups=groups,
                ins=[ar_in.opt()], outs=[ar_out.opt()])

    with tile.TileContext(nc) as tc, contextlib.ExitStack() as top:
        singles = top.enter_context(tc.tile_pool(name="singles", bufs=1))
        dram = top.enter_context(tc.tile_pool(name="dram", bufs=1, space="DRAM"))

        # ---- resident weights / inputs ----
        # Emission order matters: the in-order DMA queues must deliver wxs/wfx
        # and the first x super-tile before anything else so PE starts ASAP.
        wxs_sb = singles.tile([128, 2, HG], BF)
        wfx_sb = singles.tile([128, 2, HD], BF)
        wxs_re = wxs.ap().rearrange("(k p) n -> p k n", p=128)
        wfx_re = wfx.ap().rearrange("(k p) n -> p k n", p=128)
        # k0 chunks first so the very first matmuls can start sooner
        nc.sync.dma_start(out=wxs_sb[:, 0, :], in_=wxs_re[:, 0, :])
        ident = singles.tile([128, 128], BF)
        warm = singles.tile([1, 1], F32)
        nc.vector.memset(warm[:], 0.0)
        nc.scalar.activation(out=warm[:], in_=warm[:], func=AF.Exp)
        make_identity(nc, ident[:])
        # stacked eye(64)s: lets transposes consume partition-64-based [64,64]
        # sources directly (identity operand must share the source's base)
        ident2 = singles.tile([128, 64], BF)
        make_identity(nc, ident2[0:64, :])
        make_identity(nc, ident2[64:128, :])

        # tiles for deferred loads (DMAs emitted after the first super-tile)
        snt_sb = singles.tile([128, 6, MH], BF)
        wsp_sb = singles.tile([128, 6, HD], BF)
        w64 = singles.tile([64, 5, D], BF)  # wq,wk,wv,wck,wcv
        wo_sb = singles.tile([64, H, C], BF)

        def load_phaseB_weights():
            nc.sync.dma_start(out=snt_sb[:],
                              in_=snt.ap().rearrange("(k p) m -> p k m", p=128))
            nc.sync.dma_start(out=wsp_sb[:],
                              in_=wsp.ap().rearrange("(k p) n -> p k n", p=128))
            nc.sync.dma_start(out=w64[:], in_=w5.ap())
            nc.sync.dma_start(out=wo_sb[:],
                              in_=wo.ap().rearrange("(h d) c -> d h c", d=64))

        # eh^T: chunk c holds heads 2c,2c+1 stacked on partitions; fully
        # resident in SBUF as bf16 (4 * T * 2 bytes per partition).
        eT = singles.tile([128, 4, NT * 128], BF)
        # OS (slice-token outputs @ Wo), chunk-packed like eT
        os_sb = singles.tile([128, 4, C], BF)

        # (fx | 1) pooling rhs: manual 3-buffer rotation so the ones column
        # is preset exactly once per buffer.
        fxe_bufs = [singles.tile([128, 2, H, D + 1], BF, name=f"fxe{k}")
                    for k in range(4)]
        for fb in fxe_bufs:
            nc.vector.memset(fb[:, :, :, D], 1.0)

        # small per-partition bias columns
        bqv_c = singles.tile([64, 3], F32)
        nc.sync.dma_start(out=bqv_c[:], in_=bqv.ap().rearrange("q d -> d q"))
        bck_c = singles.tile([64, 1], F32)
        nc.sync.dma_start(out=bck_c[:], in_=bck.ap().rearrange("(d o) -> d o", o=1))
        bsp_c = singles.tile([128, 4], F32)
        nc.sync.dma_start(out=bsp_c[:], in_=bsp.ap().rearrange("(k p) -> p k", p=128))

        if flags["bias_s"]:
            bias_s_bc = singles.tile([128, HG], F32)
            nc.sync.dma_start(out=bias_s_bc[:], in_=_bcast_ap(bias_s.ap(), 128, HG))
        if flags["bcv"]:
            bcv_bc = singles.tile([128, D], F32)
            nc.sync.dma_start(out=bcv_bc[:], in_=_bcast_ap(bcv.ap(), 128, D))
        if flags["bo"]:
            # bo/H replicated on all 128 partitions (see phase B T4: each of
            # the H unpool rows carries bo/H, and per-head eh sums to 1)
            bo_bc = singles.tile([128, C], F32)
            nc.sync.dma_start(out=bo_bc[:], in_=_bcast_ap(bo.ap(), 128, C))
            nc.vector.tensor_scalar_mul(out=bo_bc[:], in0=bo_bc[:],
                                        scalar1=1.0 / H)
        if flags["bfx"]:
            bfx_bc = singles.tile([128, D], F32)

        # sonata-side SBUF tiles (produced during phase A, consumed in phase B)
        sfT = singles.tile([64, H, MH], BF)  # sf^T [d, h, m] head-major
        ksT_all = singles.tile([64, H, MH], BF)
        NMC = MH // 128  # m-chunks on this core
        vse_all = singles.tile([128, H, NMC, D + 1], BF)
        nc.vector.memset(vse_all[:, :, :, D : D + 1], 1.0)

        # ---------------- Phase A ----------------
        LAG = 6  # tiles of software-pipeline lag (even: produce/consume pair tiles)
        with contextlib.ExitStack() as phA:
            pA = phA.enter_context(tc.tile_pool(name="pA", bufs=5))
            pAx = phA.enter_context(tc.tile_pool(name="pAx", bufs=8))
            psA = phA.enter_context(tc.tile_pool(name="psA", bufs=2, space="PSUM"))
            psAcc = phA.enter_context(tc.tile_pool(name="psAcc", bufs=1, space="PSUM"))
            psH = phA.enter_context(tc.tile_pool(name="psH", bufs=1, space="PSUM"))

            # All 8 head accumulators live on partitions 0-63 across two full
            # PSUM banks (2KB-exact per-partition stride). Only the first
            # matmul touching each bank (h==0 / h==4 at tile 0) uses
            # start=True: the bank-wide has_written clear makes heads 1-3 /
            # 5-7 of tile 0 overwrite, and all later tiles accumulate.
            pool_acc = psAcc.tile([64, H, 128], F32, name="poolacc", tag="acc")

            # --- hoisted phase-B producers: sonata projection + cross K/V ---
            # Emitted interleaved into phase A to fill engine slack.
            def unit_sf(hd):  # hd chunk = heads 2hd, 2hd+1
                ps = psH.tile([128, MH], F32, tag="hps")
                for sd in range(6):
                    nc.tensor.matmul(
                        ps[:], lhsT=wsp_sb[:, sd, hd * 128 : (hd + 1) * 128],
                        rhs=snt_sb[:, sd, :], start=(sd == 0), stop=(sd == 5))
                up = pA.tile([128, MH], BF, tag="sfup")
                if flags["bsp"]:
                    nc.scalar.activation(out=sfT[:, 2 * hd, :], in_=ps[0:64, :],
                                         func=AF.Identity,
                                         bias=bsp_c[0:64, hd : hd + 1])
                    nc.scalar.activation(out=up[64:128, :], in_=ps[64:128, :],
                                         func=AF.Identity,
                                         bias=bsp_c[64:128, hd : hd + 1])
                else:
                    nc.vector.tensor_copy(out=sfT[:, 2 * hd, :], in_=ps[0:64, :])
                    nc.vector.tensor_copy(out=up[64:128, :], in_=ps[64:128, :])
                nc.gpsimd.dma_start(out=sfT[:, 2 * hd + 1, :], in_=up[64:128, :])

            def unit_kv(h):
                kp = psH.tile([64, MH], F32, tag="hps")
                nc.tensor.matmul(kp[:], lhsT=w64[:, 3, :], rhs=sfT[:, h, :],
                                 start=True, stop=True)
                nc.scalar.activation(out=ksT_all[:, h, :], in_=kp[:],
                                     func=AF.Identity, bias=bck_c[:])
                vp4 = psH.tile([128, NMC, 128], F32, tag="hps")
                for mo in range(NMC):
                    nc.tensor.matmul(vp4[:, mo, 0:64],
                                     lhsT=sfT[:, h, mo * 128 : (mo + 1) * 128],
                                     rhs=w64[:, 4, :], start=True, stop=True)
                if flags["bcv"]:
                    bcv4 = bass.AP(tensor=bcv_bc[:].tensor, offset=bcv_bc[:].offset,
                                   ap=[bcv_bc[:].ap[0], [0, NMC], [1, D]])
                    nc.vector.tensor_add(out=vse_all[:, h, :, 0:D],
                                         in0=vp4[:, :, 0:64], in1=bcv4)
                else:
                    nc.vector.tensor_copy(out=vse_all[:, h, :, 0:D],
                                          in_=vp4[:, :, 0:64])

            units = [lambda hd=hd: unit_sf(hd) for hd in range(4)]
            units += [lambda h=h: unit_kv(h) for h in range(H)]
            # spread units over mid-phase tiles (needs weights from si==0 DMAs)
            if NT >= 52:
                unit_at = {16 + (3 * k) // 2 * 2: k for k in range(len(units))}
            else:
                unit_at = {}
            units_done = [False] * len(units)

            handles = {}

            def produce2(i0, xt_sb, j0):
                # tiles i0, i0+1 batched: one ACT/DVE/Pool op per stage pair
                # amortizes the fixed memory-access cost of each instruction.
                s_ps = psA.tile([128, 2, HG], F32, tag="s", bufs=1)
                fx_ps = psA.tile([128, 2, HD], F32, tag="fx", bufs=1)
                for t in range(2):
                    tok = slice((j0 + t) * 128, (j0 + t + 1) * 128)
                    nc.tensor.matmul(s_ps[:, t, :], lhsT=xt_sb[:, 0, tok],
                                     rhs=wxs_sb[:, 0, :], start=True, stop=False)
                    nc.tensor.matmul(s_ps[:, t, :], lhsT=xt_sb[:, 1, tok],
                                     rhs=wxs_sb[:, 1, :], start=False, stop=True)
                for t in range(2):
                    tok = slice((j0 + t) * 128, (j0 + t + 1) * 128)
                    nc.tensor.matmul(fx_ps[:, t, :], lhsT=xt_sb[:, 0, tok],
                                     rhs=wfx_sb[:, 0, :], start=True, stop=False)
                    nc.tensor.matmul(fx_ps[:, t, :], lhsT=xt_sb[:, 1, tok],
                                     rhs=wfx_sb[:, 1, :], start=False, stop=True)

                e2 = pA.tile([128, 2, H, G], BF, tag="e")
                eh2 = pA.tile([128, 2, H, G], BF, tag="eh", bufs=11)
                z2 = pA.tile([128, 2, H], F32, tag="z")
                fxe2 = fxe_bufs[(i0 // 2) % len(fxe_bufs)]
                # Steady state: pair-batched ops, normalize on gpsimd (ACT/DVE
                # are the pacers). Pipeline-fill and drain batches: per-tile
                # ops on the shorter DVE path — their chain latency is exposed.
                split = False
                dve_tt = i0 < 2 or i0 >= NT - 4
                for ts in ([slice(t, t + 1) for t in range(2)]
                           if split else [slice(0, 2)]):
                    nts = ts.stop - ts.start
                    ef = e2[:, ts, :, :].rearrange("p t a b -> p (t a b)")
                    if flags["bias_s"]:
                        bias2 = bass.AP(tensor=bias_s_bc[:].tensor,
                                        offset=bias_s_bc[:].offset,
                                        ap=[bias_s_bc[:].ap[0], [0, nts], [1, HG]])
                        s_sb = pA.tile([128, 2, HG], F32, tag="ssb")
                        nc.vector.tensor_add(out=s_sb[:, ts, :],
                                             in0=s_ps[:, ts, :], in1=bias2)
                        nc.scalar.activation(out=ef, in_=s_sb[:, ts, :].rearrange(
                            "p t n -> p (t n)"), func=AF.Exp)
                    else:
                        nc.scalar.activation(
                            out=ef, in_=s_ps[:, ts, :].rearrange("p t n -> p (t n)"),
                            func=AF.Exp)

                    nc.vector.reduce_sum(out=z2[:, ts, :], in_=e2[:, ts, :, :],
                                         axis=mybir.AxisListType.X)
                    zsl = z2[:, ts, :]
                    nc.vector.reciprocal(
                        out=zsl.rearrange("p t a -> p (t a)"),
                        in_=zsl.rearrange("p t a -> p (t a)"))
                    zrb = bass.AP(tensor=zsl.tensor, offset=zsl.offset,
                                  ap=[zsl.ap[0], [H, nts], [1, H], [0, G]])
                    if dve_tt:
                        nc.vector.tensor_tensor(out=eh2[:, ts, :, :],
                                                in0=e2[:, ts, :, :], in1=zrb,
                                                op=mybir.AluOpType.mult)
                    else:
                        nc.gpsimd.tensor_tensor(out=eh2[:, ts, :, :],
                                                in0=e2[:, ts, :, :], in1=zrb,
                                                op=mybir.AluOpType.mult)
                    # pooling rhs = (fx | 1), ones columns preset per buffer
                    nc.scalar.activation(
                        out=fxe2[:, ts, :, 0:D],
                        in_=fx_ps[:, ts, :].rearrange("p t (a b) -> p t a b", a=H),
                        func=AF.Copy)
                handles[i0] = (eh2, fxe2)

            deferred = []

            def consume2(i0, defer=False):
                eh2, fxe2 = handles.pop(i0)
                for t in range(2):
                    i = i0 + t
                    for h in range(H):
                        nc.tensor.matmul(
                            pool_acc[0:64, h, 0 : D + 1],
                            lhsT=eh2[:, t, h, :], rhs=fxe2[:, t, h, :],
                            start=(i == 0 and h % 4 == 0), stop=(i == NT - 1),
                            skip_group_check=True)
                if defer:
                    # eh^T production does not gate the AllReduce — emit it
                    # after the AR staging to fill the collective round trip.
                    deferred.append((i0, eh2))
                    return
                transpose_out(i0, eh2)

            def transpose_out(i0, eh2):
                etp = psA.tile([128, 2, 4, 128], BF, tag="etp", bufs=1)
                ehf = eh2.rearrange("p t a b -> p t (a b)")
                for t in range(2):
                    for cc in range(4):
                        nc.tensor.transpose(etp[:, t, cc, :],
                                            ehf[:, t, cc * 128 : (cc + 1) * 128],
                                            ident[:])
                nc.vector.tensor_copy(
                    out=eT[:, :, i0 * 128 : (i0 + 2) * 128].rearrange(
                        "p c (t k) -> p c t k", t=2),
                    in_=etp.rearrange("p t c k -> p c t k"))

            xt_re = xt.ap().rearrange("(k p) n -> p k n", p=128)
            # Prefetch every x super-tile up front: the SP queue then serves
            # the eh^T transposes without ever blocking an x load behind them.
            xt_tiles = []
            for si in range(NSUP):
                xt_sb = pAx.tile([128, 2, 1024], BF, tag="xt", name=f"xt{si}")
                sl = slice(si * 1024, (si + 1) * 1024)
                if si == 0:
                    # finest-grained first loads: the tile-0 matmuls only need
                    # the first 256 tokens of each k-chunk
                    nc.sync.dma_start(out=xt_sb[:, 0, 0:256], in_=xt_re[:, 0, 0:256])
                    nc.sync.dma_start(out=wfx_sb[:, 0, :], in_=wfx_re[:, 0, :])
                    nc.sync.dma_start(out=xt_sb[:, 0, 256:1024],
                                      in_=xt_re[:, 0, slice(256, 1024)])
                    nc.sync.dma_start(out=wxs_sb[:, 1, :], in_=wxs_re[:, 1, :])
                    nc.sync.dma_start(out=xt_sb[:, 1, 0:256], in_=xt_re[:, 1, 0:256])
                    nc.sync.dma_start(out=wfx_sb[:, 1, :], in_=wfx_re[:, 1, :])
                    nc.sync.dma_start(out=xt_sb[:, 1, 256:1024],
                                      in_=xt_re[:, 1, slice(256, 1024)])
                else:
                    nc.sync.dma_start(out=xt_sb[:], in_=xt_re[:, :, sl])
                xt_tiles.append(xt_sb)
            for si in range(NSUP):
                xt_sb = xt_tiles[si]
                for j in range(0, 8, 2):
                    i = si * 8 + j
                    produce2(i, xt_sb, j)
                    if i >= LAG:
                        consume2(i - LAG)
                    if i in unit_at:
                        k = unit_at.pop(i)
                        units[k]()
                        units_done[k] = True
                if si == 0:
                    load_phaseB_weights()
            DEFER = 0  # deferred eh^T emission disabled (HW-divergent)
            for i in range(NT - LAG, NT, 2):
                consume2(i, defer=(i >= NT - DEFER))
            for k, u in enumerate(units):  # emit any units not yet scheduled
                if not units_done[k]:
                    u()

            # pooled partials -> AllReduce across the pair. The accumulator is
            # [64p, 8h, 65]; the AR payload (and phase B) use the pair layout
            # [128p = 64p x 2(h odd/even), 4 pairs, 65] — the DRAM staging DMA
            # applies the permutation (DRAM side is fully linear).
            pool_sb = pA.tile([64, H, D + 1], F32, tag="poolsb")
            nc.scalar.activation(out=pool_sb[:], in_=pool_acc[:, :, 0 : D + 1],
                                 func=AF.Copy)
            ar_in = dram.tile([128, 4 * (D + 1)], F32)
            ar_out = dram.tile([128, 4 * (D + 1)], F32)
            ar_in_ap = ar_in[:]
            ar_in_perm = bass.AP(
                tensor=ar_in_ap.tensor, offset=ar_in_ap.offset,
                # lockstep with pool_sb [64p][4 hp][2 hh][65]: dram row
                # p + 64*hh, column hp*65 + b
                ap=[[4 * (D + 1), 64], [D + 1, 4], [64 * 4 * (D + 1), 2], [1, D + 1]])
            nc.sync.dma_start(
                out=ar_in_perm,
                in_=pool_sb.rearrange("p (a c) b -> p a c b", a=4))
            all_reduce(ar_in, ar_out)

        # ---------------- Phase B ----------------
        with contextlib.ExitStack() as phB:
            pBw = phB.enter_context(tc.tile_pool(name="pBw", bufs=1))
            pB = phB.enter_context(tc.tile_pool(name="pB", bufs=2))
            pBh = phB.enter_context(tc.tile_pool(name="pBh", bufs=8))
            psB = phB.enter_context(tc.tile_pool(name="psB", bufs=2, space="PSUM"))
            psBs = phB.enter_context(tc.tile_pool(name="psBs", bufs=1, space="PSUM"))

            pool_red = pB.tile([128, 4, D + 1], F32, tag="poolred")
            nc.sync.dma_start(out=pool_red.rearrange("p a b -> p (a b)"), in_=ar_out[:])

            # Stage-interleaved emission: each stage is emitted for all heads
            # before the next stage, so the in-order engine queues overlap the
            # independent per-head chains instead of running them serially.
            ocst = pBw.tile([64, H, D + 1], F32)  # cross-attn partials, h-major
            osT_all = pBw.tile([64, H, 64], BF)  # self-attn out^T per head
            heads = [(hp, hh) for hp in range(4) for hh in range(2)]
            st2s, stTs, qkvTs = {}, {}, {}
            d_a, d_ea, d_za, d_pa, d_vsb, d_pat, d_ecT, d_oc = ({} for _ in range(8))

            for hp in range(4):  # S1: slice-token normalize
                pr = pool_red[:, hp, :]  # [128, 65]: heads 2hp (low), 2hp+1 (hi)
                nrm = pBh.tile([128, 1], F32, tag="nrm")
                nc.vector.tensor_scalar_add(out=nrm[:], in0=pr[:, D : D + 1],
                                            scalar1=1e-5)
                nc.vector.reciprocal(out=nrm[:], in_=nrm[:])
                st2 = pBh.tile([128, D], BF, tag="st2")
                if flags["bfx"]:
                    for hh in range(2):
                        h = 2 * hp + hh
                        sl = slice(hh * 64, hh * 64 + 64)
                        nc.sync.dma_start(out=bfx_bc[sl, :],
                                          in_=_bcast_ap(bfx.ap(), 64, D, offset=h * D))
                    tmpb = pBh.tile([128, D], F32, tag="tmpb")
                    nc.vector.tensor_scalar_mul(out=tmpb[:], in0=bfx_bc[:],
                                                scalar1=pr[:, D : D + 1])
                    nc.vector.tensor_add(out=tmpb[:], in0=tmpb[:], in1=pr[:, 0:D])
                    nc.vector.tensor_scalar_mul(out=st2[:], in0=tmpb[:], scalar1=nrm[:])
                else:
                    nc.vector.tensor_scalar_mul(out=st2[:], in0=pr[:, 0:D],
                                                scalar1=nrm[:])
                st2s[hp] = st2
            for hp in range(4):  # S2: transpose slice tokens
                stT_ps = psBs.tile([64, 128], BF, tag="small", bufs=4)
                nc.tensor.transpose(stT_ps[:], st2s[hp][:], ident[:])
                stT = pBh.tile([64, 128], BF, tag="stT")
                nc.vector.tensor_copy(out=stT[:], in_=stT_ps[:])
                stTs[hp] = stT
            for hp in range(4):  # S3: q/k/v projections (batched per pair)
                qkvTs[hp] = pBh.tile([64, 3, 128], BF, tag="qkvT",
                                     name=f"qkvT{hp}")
                qp = psBs.tile([64, 3, 128], F32, tag="small", bufs=4,
                               name=f"qp{hp}")
                for idx in range(3):
                    nc.tensor.matmul(qp[:, idx, :], lhsT=w64[:, idx, :],
                                     rhs=stTs[hp][:], start=True, stop=True)
                if flags["bqv"]:
                    for idx in range(3):
                        nc.scalar.activation(out=qkvTs[hp][:, idx, :],
                                             in_=qp[:, idx, :], func=AF.Identity,
                                             bias=bqv_c[:, idx : idx + 1])
                elif hp % 2 == 0:
                    nc.vector.tensor_copy(out=qkvTs[hp][:], in_=qp[:])
                else:
                    nc.scalar.activation(out=qkvTs[hp].rearrange("p a b -> p (a b)"),
                                         in_=qp.rearrange("p a b -> p (a b)"),
                                         func=AF.Copy)
            for hp, hh in heads:  # S4: self-attention logits
                hs = slice(hh * 64, hh * 64 + 64)
                a_ps = psBs.tile([64, 64], F32, tag="small", bufs=4)
                nc.tensor.matmul(a_ps[:], lhsT=qkvTs[hp][:, 0, hs],
                                 rhs=qkvTs[hp][:, 1, hs], start=True, stop=True)
                d_a[(hp, hh)] = a_ps
            for hp, hh in heads:  # S5: softmax exp, then row sums on DVE
                ea = pBh.tile([64, 64], F32, tag="ea")
                nc.scalar.activation(out=ea[:], in_=d_a.pop((hp, hh))[:],
                                     func=AF.Exp)
                za = pBh.tile([64, 1], F32, tag="za")
                nc.vector.reduce_sum(out=za[:], in_=ea[:],
                                     axis=mybir.AxisListType.X)
                d_ea[(hp, hh)], d_za[(hp, hh)] = ea, za
            for hp, hh in heads:  # S6: normalize attention
                za = d_za.pop((hp, hh))
                nc.vector.reciprocal(out=za[:], in_=za[:])
                pa = pBh.tile([64, 64], BF, tag="pa")
                nc.vector.tensor_scalar_mul(out=pa[:], in0=d_ea.pop((hp, hh))[:],
                                            scalar1=za[:])
                d_pa[(hp, hh)] = pa
            for hp, hh in heads:  # S7: transpose v and attention (batched)
                hs = slice(hh * 64, hh * 64 + 64)
                vp_ps = psBs.tile([64, 2, 64], BF, tag="small", bufs=4)
                nc.tensor.transpose(vp_ps[:, 0, :], qkvTs[hp][:, 2, hs],
                                    ident[0:64, 0:64])
                nc.tensor.transpose(vp_ps[:, 1, :], d_pa.pop((hp, hh))[:],
                                    ident[0:64, 0:64])
                vpat = pBh.tile([64, 2, 64], BF, tag="vpat")
                nc.vector.tensor_copy(out=vpat[:], in_=vp_ps[:])
                d_vsb[(hp, hh)] = vpat
            for hp, hh in heads:  # S8: self-attention output
                h = 2 * hp + hh
                vpat = d_vsb.pop((hp, hh))
                osf_ps = psBs.tile([64, 64], F32, tag="small", bufs=4)
                nc.tensor.matmul(osf_ps[:], lhsT=vpat[:, 0, :],
                                 rhs=vpat[:, 1, :], start=True, stop=True)
                nc.vector.tensor_copy(out=osT_all[:, h, :], in_=osf_ps[:])
            for hp, hh in heads:  # S9: cross-attention logits
                h = 2 * hp + hh
                ct_ps = psB.tile([128, NMC, 64], F32, tag="ct", bufs=2)
                for mo in range(NMC):
                    nc.tensor.matmul(ct_ps[:, mo, :],
                                     lhsT=ksT_all[:, h, mo * 128 : (mo + 1) * 128],
                                     rhs=osT_all[:, h, :], start=True, stop=True)
                d_a[(hp, hh)] = ct_ps
            for hp, hh in heads:  # S10: cross-attention exp
                ecT = pBh.tile([128, NMC, 64], BF, tag="ecT")
                ct_ps = d_a.pop((hp, hh))
                nc.scalar.activation(out=ecT.rearrange("p a b -> p (a b)"),
                                     in_=ct_ps.rearrange("p a b -> p (a b)"),
                                     func=AF.Exp)
                d_ecT[(hp, hh)] = ecT
            for hp, hh in heads:  # S11: cross numerator/denominator partials
                h = 2 * hp + hh
                ecT = d_ecT.pop((hp, hh))
                oc_ps = psBs.tile([64, 128], F32, tag="small", bufs=4)
                for mo in range(NMC):
                    nc.tensor.matmul(oc_ps[:, 0 : D + 1], lhsT=ecT[:, mo, :],
                                     rhs=vse_all[:, h, mo, :],
                                     start=(mo == 0), stop=(mo == NMC - 1))
                d_oc[(hp, hh)] = oc_ps
            for hp, hh in heads:  # S12: pack for the pair AllReduce
                h = 2 * hp + hh
                nc.vector.tensor_copy(out=ocst[0:64, h, :],
                                      in_=d_oc.pop((hp, hh))[:, 0 : D + 1])

            ar2_in = dram.tile([128, 4 * (D + 1)], F32)
            ar2_out = dram.tile([128, 4 * (D + 1)], F32)
            ar2_ap = ar2_in[:]
            ar2_perm = bass.AP(
                tensor=ar2_ap.tensor, offset=ar2_ap.offset,
                ap=[[4 * (D + 1), 64], [D + 1, 4], [64 * 4 * (D + 1), 2], [1, D + 1]])
            nc.sync.dma_start(out=ar2_perm,
                               in_=ocst.rearrange("p (a c) b -> p a c b", a=4))
            all_reduce(ar2_in, ar2_out)
            ocred = pB.tile([128, 4, D + 1], F32, tag="ocred")
            nc.sync.dma_start(out=ocred.rearrange("p a b -> p (a b)"), in_=ar2_out[:])

            # finish cross-attention + OS, stage-interleaved across heads
            oc2s, osfTs = {}, {}
            for hp in range(4):  # T1: cross-softmax normalize
                oc2 = pBh.tile([128, D], BF, tag="oc2")
                zc = pBh.tile([128, 1], F32, tag="zc")
                nc.vector.reciprocal(out=zc[:], in_=ocred[:, hp, D : D + 1])
                nc.vector.tensor_scalar_mul(out=oc2[:], in0=ocred[:, hp, 0:D],
                                            scalar1=zc[:])
                oc2s[hp] = oc2
            for hp, hh in heads:  # T2: transpose + residual add
                h = 2 * hp + hh
                src = oc2s[hp][0:64, :] if hh == 0 else oc2s[hp][64:128, :]
                idn = ident2[0:64, :] if hh == 0 else ident2[64:128, :]
                ocT_ps = psBs.tile([64, 64], BF, tag="small", bufs=4)
                nc.tensor.transpose(ocT_ps[:], src, idn)
                osfT = pBh.tile([64, 64], BF, tag="osfT")
                nc.vector.tensor_add(out=osfT[:], in0=ocT_ps[:],
                                     in1=osT_all[:, h, :])
                osfTs[(hp, hh)] = osfT
            osps = {}
            for hp, hh in heads:  # T3: project through Wo (pair shares a bank)
                h = 2 * hp + hh
                if hh == 0:
                    osps[hp] = psBs.tile([128, 512], F32, tag="osp", bufs=2,
                                         name=f"osp{hp}")
                nc.tensor.matmul(osps[hp][hh * 64 : hh * 64 + 64, 0:C],
                                 lhsT=osfTs.pop((hp, hh))[:],
                                 rhs=wo_sb[:, h, :], start=True, stop=True)
            for hp in range(4):  # T4: pack os_sb in one copy per pair
                osr = osps.pop(hp)
                if flags["bo"]:
                    nc.vector.tensor_add(out=os_sb[:, hp, :], in0=osr[:, 0:C],
                                         in1=bo_bc[:])
                else:
                    nc.vector.tensor_copy(out=os_sb[:, hp, :], in_=osr[:, 0:C])

        # ---------------- Phase C ----------------
        with contextlib.ExitStack() as phC:
            pC = phC.enter_context(tc.tile_pool(name="pC", bufs=2))
            psC = phC.enter_context(tc.tile_pool(name="psC", bufs=6, space="PSUM"))
            y_re = y.ap().rearrange("(s j p) c -> s p j c", j=8, p=128)
            for si in range(NSUP):
                stg = pC.tile([128, 8, C], BF, tag="stg")
                for j in range(8):
                    i = si * 8 + j
                    o_ps = psC.tile([128, C], F32, tag="o")
                    for cc in range(4):
                        nc.tensor.matmul(o_ps[:],
                                         lhsT=eT[:, cc, i * 128 : (i + 1) * 128],
                                         rhs=os_sb[:, cc, :],
                                         start=(cc == 0), stop=(cc == 3))
                    if i % 2 == 0:
                        nc.scalar.activation(out=stg[:, j, :], in_=o_ps[:],
                                             func=AF.Copy)
                    else:
                        nc.vector.tensor_copy(out=stg[:, j, :], in_=o_ps[:])
                nc.sync.dma_start(out=y_re[si], in_=stg[:])

    nc.compile()
    return nc


_CACHE: dict = {}


def _get_nc(n_cores: int, T: int, flags_key: tuple):
    key = (n_cores, T, flags_key)
    if key not in _CACHE:
        flags = dict(zip(("bias_s", "bqv", "bsp", "bck", "bfx", "bcv", "bo"), flags_key))
        _CACHE[key] = _build(n_cores, T, flags)
    return _CACHE[key]


def prep_inputs(inputs: dict, n_cores: int, T: int):
    """Host-side prep: transposes, weight folding, bf16 casts, per-core maps."""
    f32 = np.float32
    x = np.asarray(inputs["x"], f32)
    snt = np.asarray(inputs["sonata_features"], f32)
    temp = np.asarray(inputs["temperature"], f32).reshape(H)
    Wx, bx = np.asarray(inputs["Wx"], f32), np.asarray(inputs["bx"], f32)
    Wfx, bfx = np.asarray(inputs["Wfx"], f32), np.asarray(inputs["bfx"], f32)
    Wsl, bsl = np.asarray(inputs["Wslice"], f32), np.asarray(inputs["bslice"], f32)
    Wq, bq = np.asarray(inputs["Wq"], f32), np.asarray(inputs["bq"], f32)
    Wk, bk = np.asarray(inputs["Wk"], f32), np.asarray(inputs["bk"], f32)
    Wv, bv = np.asarray(inputs["Wv"], f32), np.asarray(inputs["bv"], f32)
    Wsp, bsp = np.asarray(inputs["Wsp"], f32), np.asarray(inputs["bsp"], f32)
    Wck, bck = np.asarray(inputs["Wck"], f32), np.asarray(inputs["bck"], f32)
    Wcv, bcv = np.asarray(inputs["Wcv"], f32), np.asarray(inputs["bcv"], f32)
    Wo, bo = np.asarray(inputs["Wo"], f32), np.asarray(inputs["bo"], f32)

    Wxs = np.zeros((C, HG), f32)
    bias_s = np.zeros((HG,), f32)
    for h in range(H):
        Wxs[:, h * G : (h + 1) * G] = (Wx[:, h * D : (h + 1) * D] @ Wsl) / temp[h]
        bias_s[h * G : (h + 1) * G] = (bx[h * D : (h + 1) * D] @ Wsl + bsl) / temp[h]
    flags = {
        "bias_s": bool(np.any(bias_s != 0)),
        "bqv": bool(np.any(bq != 0) or np.any(bk != 0) or np.any(bv != 0)),
        "bsp": bool(np.any(bsp != 0)),
        "bck": bool(np.any(bck != 0)),
        "bfx": bool(np.any(bfx != 0)),
        "bcv": bool(np.any(bcv != 0)),
        "bo": bool(np.any(bo != 0)),
    }
    w5 = np.stack([Wq * SCALE, Wk, Wv, Wck * SCALE, Wcv], axis=1)  # [D, 5, D]
    shared = {
        "wxs": np.ascontiguousarray(Wxs).astype(NPBF),
        "wfx": np.ascontiguousarray(Wfx).astype(NPBF),
        "wsp": np.ascontiguousarray(Wsp).astype(NPBF),
        "w5": np.ascontiguousarray(w5).astype(NPBF),
        "wo": np.ascontiguousarray(Wo).astype(NPBF),
        "bqv": np.ascontiguousarray(np.stack([bq * SCALE, bk, bv])),
        "bck": np.ascontiguousarray(bck * SCALE),
        "bsp": np.ascontiguousarray(bsp),
    }
    if flags["bias_s"]:
        shared["bias_s"] = bias_s
    if flags["bfx"]:
        shared["bfx"] = bfx
    if flags["bcv"]:
        shared["bcv"] = bcv
    if flags["bo"]:
        shared["bo"] = bo

    in_maps = []
    for c in range(n_cores):
        b, half = c // 2, c % 2
        xt_c = np.ascontiguousarray(x[b, half * T : (half + 1) * T, :].T).astype(NPBF)
        snt_c = np.ascontiguousarray(
            snt[b].T[:, half * MH : (half + 1) * MH]).astype(NPBF)
        in_maps.append({"xt": xt_c, "snt": snt_c, **shared})
    return in_maps, flags


def run(inputs: dict, n_cores: int = 8, T: int = N // 2, **spmd_kwargs):
    in_maps, flags = prep_inputs(inputs, n_cores, T)
    nc = _get_nc(n_cores, T, tuple(flags.values()))
    res = run_bass_kernel_spmd(nc, in_maps, core_ids=list(range(n_cores)),
                               **spmd_kwargs)
    out = np.zeros((B, N, C), np.float32)
    for c in range(n_cores):
        b, half = c // 2, c % 2
        out[b, half * T : (half + 1) * T, :] = np.asarray(
            res.results[c]["y"]).astype(np.float32)
    return out, res


def kernel(**inputs) -> np.ndarray:
    out, _ = run(inputs)
    return out


# revision 52
# speedup vs baseline: 1.0120x; 1.0050x over previous
"""Trainium2 Bass kernel for Enhanced Physics Attention with Sonata.

Contract: kernel(**inputs) takes FULL unsharded numpy inputs (as produced by
setup_inputs()) and returns the FULL [B, N, C] output. Internally shards
across 8 NeuronCores: core c handles batch c//2, token half c%2, and sonata
half c%2 (cross-attention partials). Two pairwise AllReduces: pooled slice
tokens after phase A, cross-attention numerators/denominators in phase B.

Math (fp32 PSUM accumulation, bf16 operands on the hot paths; rel err ~6e-3):
  Host folds Wslice+temperature into Wxs (s = x @ Wxs), SCALE into Wq/Wck,
  and casts x/weights to bf16.
  Phase A (token-major, lag-6 produce/consume software pipeline over
    pair-batched 128-token tiles): e = exp(s) [logits bounded ~3, no max
    needed], z = grouped sum (DVE), eh = e/z (gpsimd); pooling
    slice_token = eh^T @ (fx | 1) — the appended ones column yields
    slice_norm for free; eh^T (PE transpose + DVE 2x copy) stays fully
    resident in SBUF as bf16. Sonata projections + cross K/V are emitted
    interleaved mid-phase to fill engine slack.
  Phase B (stage-interleaved across heads so the in-order engine queues
    overlap the chains): slice self-attention (replicated) + sonata
    cross-attention over this core's sonata half; ones-column gives the
    softmax denominator, the pair AllReduce sums numerator+denominator
    over the full sonata. Both heads of a pair project through Wo into one
    full-bank PSUM tile (odd head via col tile_position) so os packs in
    one DVE copy.
  Phase C: out = eh_T.T @ (out_slice @ Wo), written back as bf16.

Self-contained: hardcodes all shapes; does not read sibling files.
"""

import contextlib
import sys

try:
    import concourse  # noqa: F401
except ImportError:
    sys.path.insert(0, "/opt/trn_rl_repo")

import ml_dtypes
import numpy as np

import concourse.bass as bass
import concourse.tile as tile
from concourse import bacc, mybir
from concourse.bass_utils import run_bass_kernel_spmd
from concourse.masks import make_identity

F32 = mybir.dt.float32
BF = mybir.dt.bfloat16
NPBF = np.dtype(ml_dtypes.bfloat16)
AF = mybir.ActivationFunctionType

# Problem shapes
B, N, C = 4, 16384, 256
H, D, G = 8, 64, 64
M, SD = 1024, 768
SCALE = D**-0.5
HG = H * G  # 512
HD = H * D  # 512
MH = M // 2  # sonata tokens per core (m-split across the pair)


def _bcast_ap(dram_ap: bass.AP, parts: int, n: int, offset: int = 0) -> bass.AP:
    """AP reading n contiguous DRAM floats, replicated across `parts` partitions."""
    return bass.AP(
        tensor=dram_ap.tensor,
        offset=dram_ap.offset + offset,
        ap=[[0, parts], [1, n]],
    )


def _fbcast(ap2d: bass.AP, rep: int) -> bass.AP:
    """[p, k] AP -> [p, k, rep] with step-0 innermost broadcast."""
    return bass.AP(tensor=ap2d.tensor, offset=ap2d.offset,
                   ap=[ap2d.ap[0], ap2d.ap[1], [0, rep]])


def _build(n_cores: int, T: int, flags: dict, no_collective: bool = False):
    """Build the per-core Bass module. T = tokens per core (multiple of 1024)."""
    assert T % 1024 == 0
    NSUP = T // 1024  # super-tiles (x loads)
    NT = T // 128  # 128-token tiles

    nc = bacc.Bacc(
        "TRN2", target_bir_lowering=False, debug=False, num_devices=n_cores
    )

    # ---- DRAM I/O ----
    xt = nc.dram_tensor("xt", [C, T], BF, kind="ExternalInput")  # x slice, transposed
    snt = nc.dram_tensor("snt", [SD, MH], BF, kind="ExternalInput")  # sonata half, T
    wxs = nc.dram_tensor("wxs", [C, HG], BF, kind="ExternalInput")
    wfx = nc.dram_tensor("wfx", [C, HD], BF, kind="ExternalInput")
    wsp = nc.dram_tensor("wsp", [SD, HD], BF, kind="ExternalInput")
    w5 = nc.dram_tensor("w5", [D, 5, D], BF, kind="ExternalInput")  # q,k,v,ck,cv
    wo = nc.dram_tensor("wo", [HD, C], BF, kind="ExternalInput")
    bqv = nc.dram_tensor("bqv", [3, D], F32, kind="ExternalInput")  # bq,bk,bv rows
    bck = nc.dram_tensor("bck", [D], F32, kind="ExternalInput")
    bsp = nc.dram_tensor("bsp", [HD], F32, kind="ExternalInput")
    if flags["bias_s"]:
        bias_s = nc.dram_tensor("bias_s", [HG], F32, kind="ExternalInput")
    if flags["bfx"]:
        bfx = nc.dram_tensor("bfx", [HD], F32, kind="ExternalInput")
    if flags["bcv"]:
        bcv = nc.dram_tensor("bcv", [D], F32, kind="ExternalInput")
    if flags["bo"]:
        bo = nc.dram_tensor("bo", [C], F32, kind="ExternalInput")
    y = nc.dram_tensor("y", [T, C], BF, kind="ExternalOutput")

    groups = [[2 * i, 2 * i + 1] for i in range(n_cores // 2)]

    def all_reduce(ar_in, ar_out):
        if no_collective:
            nc.gpsimd.dma_start(out=ar_out[:], in_=ar_in[:])
        else:
            nc.gpsimd.collective_compute(
                "AllReduce", mybir.AluOpType.add, replica_groups=groups,
                ins=[ar_in.opt()], outs=[ar_out.opt()])

    with tile.TileContext(nc) as tc, contextlib.ExitStack() as top:
        singles = top.enter_context(tc.tile_pool(name="singles", bufs=1))
        dram = top.enter_context(tc.tile_pool(name="dram", bufs=1, space="DRAM"))

        # ---- resident weights / inputs ----
        # Emission order matters: the in-order DMA queues must deliver wxs/wfx
        # and the first x super-tile before anything else so PE starts ASAP.
        wxs_sb = singles.tile([128, 2, HG], BF)
        wfx_sb = singles.tile([128, 2, HD], BF)
        wxs_re = wxs.ap().rearrange("(k p) n -> p k n", p=128)
        wfx_re = wfx.ap().rearrange("(k p) n -> p k n", p=128)
        # k0 chunks first so the very first matmuls can start sooner
        nc.sync.dma_start(out=wxs_sb[:, 0, :], in_=wxs_re[:, 0, :])
        ident = singles.tile([128, 128], BF)
        warm = singles.tile([1, 1], F32)
        nc.vector.memset(warm[:], 0.0)
        nc.scalar.activation(out=warm[:], in_=warm[:], func=AF.Exp)
        make_identity(nc, ident[:])
        # stacked eye(64)s: lets transposes consume partition-64-based [64,64]
        # sources directly (identity operand must share the source's base)
        ident2 = singles.tile([128, 64], BF)
        make_identity(nc, ident2[0:64, :])
        make_identity(nc, ident2[64:128, :])

        # tiles for deferred loads (DMAs emitted after the first super-tile)
        snt_sb = singles.tile([128, 6, MH], BF)
        wsp_sb = singles.tile([128, 6, HD], BF)
        w64 = singles.tile([64, 5, D], BF)  # wq,wk,wv,wck,wcv
        wo_sb = singles.tile([64, H, C], BF)

        def load_phaseB_weights():
            nc.sync.dma_start(out=snt_sb[:],
                              in_=snt.ap().rearrange("(k p) m -> p k m", p=128))
            nc.sync.dma_start(out=wsp_sb[:],
                              in_=wsp.ap().rearrange("(k p) n -> p k n", p=128))
            nc.sync.dma_start(out=w64[:], in_=w5.ap())
            nc.sync.dma_start(out=wo_sb[:],
                              in_=wo.ap().rearrange("(h d) c -> d h c", d=64))

        # eh^T: chunk c holds heads 2c,2c+1 stacked on partitions; fully
        # resident in SBUF as bf16 (4 * T * 2 bytes per partition).
        eT = singles.tile([128, 4, NT * 128], BF)
        # OS (slice-token outputs @ Wo), chunk-packed like eT
        os_sb = singles.tile([128, 4, C], BF)

        # (fx | 1) pooling rhs: manual 3-buffer rotation so the ones column
        # is preset exactly once per buffer.
        fxe_bufs = [singles.tile([128, 2, H, D + 1], BF, name=f"fxe{k}")
                    for k in range(4)]
        for fb in fxe_bufs:
            nc.vector.memset(fb[:, :, :, D], 1.0)

        # small per-partition bias columns
        bqv_c = singles.tile([64, 3], F32)
        nc.sync.dma_start(out=bqv_c[:], in_=bqv.ap().rearrange("q d -> d q"))
        bck_c = singles.tile([64, 1], F32)
        nc.sync.dma_start(out=bck_c[:], in_=bck.ap().rearrange("(d o) -> d o", o=1))
        bsp_c = singles.tile([128, 4], F32)
        nc.sync.dma_start(out=bsp_c[:], in_=bsp.ap().rearrange("(k p) -> p k", p=128))

        if flags["bias_s"]:
            bias_s_bc = singles.tile([128, HG], F32)
            nc.sync.dma_start(out=bias_s_bc[:], in_=_bcast_ap(bias_s.ap(), 128, HG))
        if flags["bcv"]:
            bcv_bc = singles.tile([128, D], F32)
            nc.sync.dma_start(out=bcv_bc[:], in_=_bcast_ap(bcv.ap(), 128, D))
        if flags["bo"]:
            # bo/H replicated on all 128 partitions (see phase B T4: each of
            # the H unpool rows carries bo/H, and per-head eh sums to 1)
            bo_bc = singles.tile([128, C], F32)
            nc.sync.dma_start(out=bo_bc[:], in_=_bcast_ap(bo.ap(), 128, C))
            nc.vector.tensor_scalar_mul(out=bo_bc[:], in0=bo_bc[:],
                                        scalar1=1.0 / H)
        if flags["bfx"]:
            bfx_bc = singles.tile([128, D], F32)

        # sonata-side SBUF tiles (produced during phase A, consumed in phase B)
        sfT = singles.tile([64, H, MH], BF)  # sf^T [d, h, m] head-major
        ksT_all = singles.tile([64, H, MH], BF)
        NMC = MH // 128  # m-chunks on this core
        vse_all = singles.tile([128, H, NMC, D + 1], BF)
        nc.vector.memset(vse_all[:, :, :, D : D + 1], 1.0)

        # ---------------- Phase A ----------------
        LAG = 6  # tiles of software-pipeline lag (even: produce/consume pair tiles)
        with contextlib.ExitStack() as phA:
            pA = phA.enter_context(tc.tile_pool(name="pA", bufs=5))
            pAx = phA.enter_context(tc.tile_pool(name="pAx", bufs=8))
            psA = phA.enter_context(tc.tile_pool(name="psA", bufs=2, space="PSUM"))
            psAcc = phA.enter_context(tc.tile_pool(name="psAcc", bufs=1, space="PSUM"))
            psH = phA.enter_context(tc.tile_pool(name="psH", bufs=1, space="PSUM"))

            # All 8 head accumulators live on partitions 0-63 across two full
            # PSUM banks (2KB-exact per-partition stride). Only the first
            # matmul touching each bank (h==0 / h==4 at tile 0) uses
            # start=True: the bank-wide has_written clear makes heads 1-3 /
            # 5-7 of tile 0 overwrite, and all later tiles accumulate.
            pool_acc = psAcc.tile([64, H, 128], F32, name="poolacc", tag="acc")

            # --- hoisted phase-B producers: sonata projection + cross K/V ---
            # Emitted interleaved into phase A to fill engine slack.
            def unit_sf(hd):  # hd chunk = heads 2hd, 2hd+1
                ps = psH.tile([128, MH], F32, tag="hps")
                for sd in range(6):
                    nc.tensor.matmul(
                        ps[:], lhsT=wsp_sb[:, sd, hd * 128 : (hd + 1) * 128],
                        rhs=snt_sb[:, sd, :], start=(sd == 0), stop=(sd == 5))
                up = pA.tile([128, MH], BF, tag="sfup")
                if flags["bsp"]:
                    nc.scalar.activation(out=sfT[:, 2 * hd, :], in_=ps[0:64, :],
                                         func=AF.Identity,
                                         bias=bsp_c[0:64, hd : hd + 1])
                    nc.scalar.activation(out=up[64:128, :], in_=ps[64:128, :],
                                         func=AF.Identity,
                                         bias=bsp_c[64:128, hd : hd + 1])
                else:
                    nc.vector.tensor_copy(out=sfT[:, 2 * hd, :], in_=ps[0:64, :])
                    nc.vector.tensor_copy(out=up[64:128, :], in_=ps[64:128, :])
                nc.gpsimd.dma_start(out=sfT[:, 2 * hd + 1, :], in_=up[64:128, :])

            def unit_kv(h):
                kp = psH.tile([64, MH], F32, tag="hps")
                nc.tensor.matmul(kp[:], lhsT=w64[:, 3, :], rhs=sfT[:, h, :],
                                 start=True, stop=True)
                nc.scalar.activation(out=ksT_all[:, h, :], in_=kp[:],
                                     func=AF.Identity, bias=bck_c[:])
                vp4 = psH.tile([128, NMC, 128], F32, tag="hps")
                for mo in range(NMC):
                    nc.tensor.matmul(vp4[:, mo, 0:64],
                                     lhsT=sfT[:, h, mo * 128 : (mo + 1) * 128],
                                     rhs=w64[:, 4, :], start=True, stop=True)
                if flags["bcv"]:
                    bcv4 = bass.AP(tensor=bcv_bc[:].tensor, offset=bcv_bc[:].offset,
                                   ap=[bcv_bc[:].ap[0], [0, NMC], [1, D]])
                    nc.vector.tensor_add(out=vse_all[:, h, :, 0:D],
                                         in0=vp4[:, :, 0:64], in1=bcv4)
                else:
                    nc.vector.tensor_copy(out=vse_all[:, h, :, 0:D],
                                          in_=vp4[:, :, 0:64])

            units = [lambda hd=hd: unit_sf(hd) for hd in range(4)]
            units += [lambda h=h: unit_kv(h) for h in range(H)]
            # spread units over mid-phase tiles (needs weights from si==0 DMAs)
            if NT >= 52:
                unit_at = {16 + (3 * k) // 2 * 2: k for k in range(len(units))}
            else:
                unit_at = {}
            units_done = [False] * len(units)

            handles = {}

            def produce2(i0, xt_sb, j0):
                # tiles i0, i0+1 batched: one ACT/DVE/Pool op per stage pair
                # amortizes the fixed memory-access cost of each instruction.
                s_ps = psA.tile([128, 2, HG], F32, tag="s", bufs=1)
                fx_ps = psA.tile([128, 2, HD], F32, tag="fx", bufs=1)
                for t in range(2):
                    tok = slice((j0 + t) * 128, (j0 + t + 1) * 128)
                    nc.tensor.matmul(s_ps[:, t, :], lhsT=xt_sb[:, 0, tok],
                                     rhs=wxs_sb[:, 0, :], start=True, stop=False)
                    nc.tensor.matmul(s_ps[:, t, :], lhsT=xt_sb[:, 1, tok],
                                     rhs=wxs_sb[:, 1, :], start=False, stop=True)
                for t in range(2):
                    tok = slice((j0 + t) * 128, (j0 + t + 1) * 128)
                    nc.tensor.matmul(fx_ps[:, t, :], lhsT=xt_sb[:, 0, tok],
                                     rhs=wfx_sb[:, 0, :], start=True, stop=False)
                    nc.tensor.matmul(fx_ps[:, t, :], lhsT=xt_sb[:, 1, tok],
                                     rhs=wfx_sb[:, 1, :], start=False, stop=True)

                e2 = pA.tile([128, 2, H, G], BF, tag="e")
                eh2 = pA.tile([128, 2, H, G], BF, tag="eh", bufs=11)
                z2 = pA.tile([128, 2, H], F32, tag="z")
                fxe2 = fxe_bufs[(i0 // 2) % len(fxe_bufs)]
                # Steady state: pair-batched ops, normalize on gpsimd (ACT/DVE
                # are the pacers). Pipeline-fill and drain batches: per-tile
                # ops on the shorter DVE path — their chain latency is exposed.
                split = i0 == NT - 2
                dve_tt = i0 < 2 or i0 >= NT - 2
                parts = ([slice(t, t + 1) for t in range(2)]
                         if split else [slice(0, 2)])
                for ts in parts:
                    nts = ts.stop - ts.start
                    ef = e2[:, ts, :, :].rearrange("p t a b -> p (t a b)")
                    if flags["bias_s"]:
                        bias2 = bass.AP(tensor=bias_s_bc[:].tensor,
                                        offset=bias_s_bc[:].offset,
                                        ap=[bias_s_bc[:].ap[0], [0, nts], [1, HG]])
                        s_sb = pA.tile([128, 2, HG], F32, tag="ssb")
                        nc.vector.tensor_add(out=s_sb[:, ts, :],
                                             in0=s_ps[:, ts, :], in1=bias2)
                        nc.scalar.activation(out=ef, in_=s_sb[:, ts, :].rearrange(
                            "p t n -> p (t n)"), func=AF.Exp)
                    else:
                        nc.scalar.activation(
                            out=ef, in_=s_ps[:, ts, :].rearrange("p t n -> p (t n)"),
                            func=AF.Exp)
                for ts in parts:
                    nts = ts.stop - ts.start
                    nc.vector.reduce_sum(out=z2[:, ts, :], in_=e2[:, ts, :, :],
                                         axis=mybir.AxisListType.X)
                    zsl = z2[:, ts, :]
                    nc.vector.reciprocal(
                        out=zsl.rearrange("p t a -> p (t a)"),
                        in_=zsl.rearrange("p t a -> p (t a)"))
                    zrb = bass.AP(tensor=zsl.tensor, offset=zsl.offset,
                                  ap=[zsl.ap[0], [H, nts], [1, H], [0, G]])
                    if dve_tt:
                        nc.vector.tensor_tensor(out=eh2[:, ts, :, :],
                                                in0=e2[:, ts, :, :], in1=zrb,
                                                op=mybir.AluOpType.mult)
                    else:
                        nc.gpsimd.tensor_tensor(out=eh2[:, ts, :, :],
                                                in0=e2[:, ts, :, :], in1=zrb,
                                                op=mybir.AluOpType.mult)
                # pooling rhs = (fx | 1), ones columns preset per buffer
                nc.scalar.activation(
                    out=fxe2[:, :, :, 0:D],
                    in_=fx_ps.rearrange("p t (a b) -> p t a b", a=H),
                    func=AF.Copy)
                handles[i0] = (eh2, fxe2)

            deferred = []

            def consume2(i0, defer=False):
                eh2, fxe2 = handles.pop(i0)
                for t in range(2):
                    i = i0 + t
                    for h in range(H):
                        nc.tensor.matmul(
                            pool_acc[0:64, h, 0 : D + 1],
                            lhsT=eh2[:, t, h, :], rhs=fxe2[:, t, h, :],
                            start=(i == 0 and h % 4 == 0), stop=(i == NT - 1),
                            skip_group_check=True)
                if defer:
                    # eh^T production does not gate the AllReduce — emit it
                    # after the AR staging to fill the collective round trip.
                    deferred.append((i0, eh2))
                    return
                transpose_out(i0, eh2)

            def transpose_out(i0, eh2):
                etp = psA.tile([128, 2, 4, 128], BF, tag="etp", bufs=1)
                ehf = eh2.rearrange("p t a b -> p t (a b)")
                for t in range(2):
                    for cc in range(4):
                        nc.tensor.transpose(etp[:, t, cc, :],
                                            ehf[:, t, cc * 128 : (cc + 1) * 128],
                                            ident[:])
                nc.vector.tensor_copy(
                    out=eT[:, :, i0 * 128 : (i0 + 2) * 128].rearrange(
                        "p c (t k) -> p c t k", t=2),
                    in_=etp.rearrange("p t c k -> p c t k"))

            xt_re = xt.ap().rearrange("(k p) n -> p k n", p=128)
            # Prefetch every x super-tile up front: the SP queue then serves
            # the eh^T transposes without ever blocking an x load behind them.
            xt_tiles = []
            for si in range(NSUP):
                xt_sb = pAx.tile([128, 2, 1024], BF, tag="xt", name=f"xt{si}")
                sl = slice(si * 1024, (si + 1) * 1024)
                if si == 0:
                    # finest-grained first loads: the tile-0 matmuls only need
                    # the first 256 tokens of each k-chunk
                    nc.sync.dma_start(out=xt_sb[:, 0, 0:256], in_=xt_re[:, 0, 0:256])
                    nc.sync.dma_start(out=wfx_sb[:, 0, :], in_=wfx_re[:, 0, :])
                    nc.sync.dma_start(out=xt_sb[:, 0, 256:1024],
                                      in_=xt_re[:, 0, slice(256, 1024)])
                    nc.sync.dma_start(out=wxs_sb[:, 1, :], in_=wxs_re[:, 1, :])
                    nc.sync.dma_start(out=xt_sb[:, 1, 0:256], in_=xt_re[:, 1, 0:256])
                    nc.sync.dma_start(out=wfx_sb[:, 1, :], in_=wfx_re[:, 1, :])
                    nc.sync.dma_start(out=xt_sb[:, 1, 256:1024],
                                      in_=xt_re[:, 1, slice(256, 1024)])
                else:
                    nc.sync.dma_start(out=xt_sb[:], in_=xt_re[:, :, sl])
                xt_tiles.append(xt_sb)
            for si in range(NSUP):
                xt_sb = xt_tiles[si]
                for j in range(0, 8, 2):
                    i = si * 8 + j
                    produce2(i, xt_sb, j)
                    if i >= LAG:
                        consume2(i - LAG)
                    if i in unit_at:
                        k = unit_at.pop(i)
                        units[k]()
                        units_done[k] = True
                if si == 0:
                    load_phaseB_weights()
            DEFER = 0  # deferred eh^T emission disabled (HW-divergent)
            for i in range(NT - LAG, NT, 2):
                consume2(i, defer=(i >= NT - DEFER))
            for k, u in enumerate(units):  # emit any units not yet scheduled
                if not units_done[k]:
                    u()

            # pooled partials -> AllReduce across the pair. The accumulator is
            # [64p, 8h, 65]; the AR payload (and phase B) use the pair layout
            # [128p = 64p x 2(h odd/even), 4 pairs, 65] — the DRAM staging DMA
            # applies the permutation (DRAM side is fully linear).
            pool_sb = pA.tile([64, H, D + 1], F32, tag="poolsb")
            nc.scalar.activation(out=pool_sb[:], in_=pool_acc[:, :, 0 : D + 1],
                                 func=AF.Copy)
            ar_in = dram.tile([128, 4 * (D + 1)], F32)
            ar_out = dram.tile([128, 4 * (D + 1)], F32)
            ar_in_ap = ar_in[:]
            ar_in_perm = bass.AP(
                tensor=ar_in_ap.tensor, offset=ar_in_ap.offset,
                # lockstep with pool_sb [64p][4 hp][2 hh][65]: dram row
                # p + 64*hh, column hp*65 + b
                ap=[[4 * (D + 1), 64], [D + 1, 4], [64 * 4 * (D + 1), 2], [1, D + 1]])
            nc.sync.dma_start(
                out=ar_in_perm,
                in_=pool_sb.rearrange("p (a c) b -> p a c b", a=4))
            all_reduce(ar_in, ar_out)

        # ---------------- Phase B ----------------
        with contextlib.ExitStack() as phB:
            pBw = phB.enter_context(tc.tile_pool(name="pBw", bufs=1))
            pB = phB.enter_context(tc.tile_pool(name="pB", bufs=2))
            pBh = phB.enter_context(tc.tile_pool(name="pBh", bufs=8))
            psB = phB.enter_context(tc.tile_pool(name="psB", bufs=2, space="PSUM"))
            psBs = phB.enter_context(tc.tile_pool(name="psBs", bufs=1, space="PSUM"))

            pool_red = pB.tile([128, 4, D + 1], F32, tag="poolred")
            nc.sync.dma_start(out=pool_red.rearrange("p a b -> p (a b)"), in_=ar_out[:])

            # Stage-interleaved emission: each stage is emitted for all heads
            # before the next stage, so the in-order engine queues overlap the
            # independent per-head chains instead of running them serially.
            ocst = pBw.tile([64, H, D + 1], F32)  # cross-attn partials, h-major
            osT_all = pBw.tile([64, H, 64], BF)  # self-attn out^T per head
            heads = [(hp, hh) for hp in range(4) for hh in range(2)]
            st2s, stTs, qkvTs = {}, {}, {}
            d_a, d_ea, d_za, d_pa, d_vsb, d_pat, d_ecT, d_oc = ({} for _ in range(8))

            for hp in range(4):  # S1: slice-token normalize
                pr = pool_red[:, hp, :]  # [128, 65]: heads 2hp (low), 2hp+1 (hi)
                nrm = pBh.tile([128, 1], F32, tag="nrm")
                nc.vector.tensor_scalar_add(out=nrm[:], in0=pr[:, D : D + 1],
                                            scalar1=1e-5)
                nc.vector.reciprocal(out=nrm[:], in_=nrm[:])
                st2 = pBh.tile([128, D], BF, tag="st2")
                if flags["bfx"]:
                    for hh in range(2):
                        h = 2 * hp + hh
                        sl = slice(hh * 64, hh * 64 + 64)
                        nc.sync.dma_start(out=bfx_bc[sl, :],
                                          in_=_bcast_ap(bfx.ap(), 64, D, offset=h * D))
                    tmpb = pBh.tile([128, D], F32, tag="tmpb")
                    nc.vector.tensor_scalar_mul(out=tmpb[:], in0=bfx_bc[:],
                                                scalar1=pr[:, D : D + 1])
                    nc.vector.tensor_add(out=tmpb[:], in0=tmpb[:], in1=pr[:, 0:D])
                    nc.vector.tensor_scalar_mul(out=st2[:], in0=tmpb[:], scalar1=nrm[:])
                else:
                    nc.vector.tensor_scalar_mul(out=st2[:], in0=pr[:, 0:D],
                                                scalar1=nrm[:])
                st2s[hp] = st2
            for hp in range(4):  # S2: transpose slice tokens
                stT_ps = psBs.tile([64, 128], BF, tag="small", bufs=4)
                nc.tensor.transpose(stT_ps[:], st2s[hp][:], ident[:])
                stT = pBh.tile([64, 128], BF, tag="stT")
                nc.vector.tensor_copy(out=stT[:], in_=stT_ps[:])
                stTs[hp] = stT
            for hp in range(4):  # S3: q/k/v projections (batched per pair)
                qkvTs[hp] = pBh.tile([64, 3, 128], BF, tag="qkvT",
                                     name=f"qkvT{hp}")
                qp = psBs.tile([64, 3, 128], F32, tag="small", bufs=4,
                               name=f"qp{hp}")
                for idx in range(3):
                    nc.tensor.matmul(qp[:, idx, :], lhsT=w64[:, idx, :],
                                     rhs=stTs[hp][:], start=True, stop=True)
                if flags["bqv"]:
                    for idx in range(3):
                        nc.scalar.activation(out=qkvTs[hp][:, idx, :],
                                             in_=qp[:, idx, :], func=AF.Identity,
                                             bias=bqv_c[:, idx : idx + 1])
                elif hp % 2 == 0:
                    nc.vector.tensor_copy(out=qkvTs[hp][:], in_=qp[:])
                else:
                    nc.scalar.activation(out=qkvTs[hp].rearrange("p a b -> p (a b)"),
                                         in_=qp.rearrange("p a b -> p (a b)"),
                                         func=AF.Copy)
            for hp, hh in heads:  # S4: self-attention logits
                hs = slice(hh * 64, hh * 64 + 64)
                a_ps = psBs.tile([64, 64], F32, tag="small", bufs=4)
                nc.tensor.matmul(a_ps[:], lhsT=qkvTs[hp][:, 0, hs],
                                 rhs=qkvTs[hp][:, 1, hs], start=True, stop=True)
                d_a[(hp, hh)] = a_ps
            for hp, hh in heads:  # S5: softmax exp, then row sums on DVE
                ea = pBh.tile([64, 64], F32, tag="ea")
                nc.scalar.activation(out=ea[:], in_=d_a.pop((hp, hh))[:],
                                     func=AF.Exp)
                za = pBh.tile([64, 1], F32, tag="za")
                nc.vector.reduce_sum(out=za[:], in_=ea[:],
                                     axis=mybir.AxisListType.X)
                d_ea[(hp, hh)], d_za[(hp, hh)] = ea, za
            for hp, hh in heads:  # S6: normalize attention
                za = d_za.pop((hp, hh))
                nc.vector.reciprocal(out=za[:], in_=za[:])
                pa = pBh.tile([64, 64], BF, tag="pa")
                nc.vector.tensor_scalar_mul(out=pa[:], in0=d_ea.pop((hp, hh))[:],
                                            scalar1=za[:])
                d_pa[(hp, hh)] = pa
            for hp, hh in heads:  # S7: transpose v and attention (batched)
                hs = slice(hh * 64, hh * 64 + 64)
                vp_ps = psBs.tile([64, 2, 64], BF, tag="small", bufs=4)
                nc.tensor.transpose(vp_ps[:, 0, :], qkvTs[hp][:, 2, hs],
                                    ident[0:64, 0:64])
                nc.tensor.transpose(vp_ps[:, 1, :], d_pa.pop((hp, hh))[:],
                                    ident[0:64, 0:64])
                vpat = pBh.tile([64, 2, 64], BF, tag="vpat")
                nc.vector.tensor_copy(out=vpat[:], in_=vp_ps[:])
                d_vsb[(hp, hh)] = vpat
            for hp, hh in heads:  # S8: self-attention output
                h = 2 * hp + hh
                vpat = d_vsb.pop((hp, hh))
                osf_ps = psBs.tile([64, 64], F32, tag="small", bufs=4)
                nc.tensor.matmul(osf_ps[:], lhsT=vpat[:, 0, :],
                                 rhs=vpat[:, 1, :], start=True, stop=True)
                nc.vector.tensor_copy(out=osT_all[:, h, :], in_=osf_ps[:])
            for hp, hh in heads:  # S9: cross-attention logits
                h = 2 * hp + hh
                ct_ps = psB.tile([128, NMC, 64], F32, tag="ct", bufs=2)
                for mo in range(NMC):
                    nc.tensor.matmul(ct_ps[:, mo, :],
                                     lhsT=ksT_all[:, h, mo * 128 : (mo + 1) * 128],
                                     rhs=osT_all[:, h, :], start=True, stop=True)
                d_a[(hp, hh)] = ct_ps
            for hp, hh in heads:  # S10: cross-attention exp
                ecT = pBh.tile([128, NMC, 64], BF, tag="ecT")
                ct_ps = d_a.pop((hp, hh))
                nc.scalar.activation(out=ecT.rearrange("p a b -> p (a b)"),
                                     in_=ct_ps.rearrange("p a b -> p (a b)"),
                                     func=AF.Exp)
                d_ecT[(hp, hh)] = ecT
            for hp, hh in heads:  # S11: cross numerator/denominator partials
                h = 2 * hp + hh
                ecT = d_ecT.pop((hp, hh))
                oc_ps = psBs.tile([64, 128], F32, tag="small", bufs=4)
                for mo in range(NMC):
                    nc.tensor.matmul(oc_ps[:, 0 : D + 1], lhsT=ecT[:, mo, :],
                                     rhs=vse_all[:, h, mo, :],
                                     start=(mo == 0), stop=(mo == NMC - 1))
                d_oc[(hp, hh)] = oc_ps
            for hp, hh in heads:  # S12: pack for the pair AllReduce
                h = 2 * hp + hh
                nc.vector.tensor_copy(out=ocst[0:64, h, :],
                                      in_=d_oc.pop((hp, hh))[:, 0 : D + 1])

            ar2_in = dram.tile([128, 4 * (D + 1)], F32)
            ar2_out = dram.tile([128, 4 * (D + 1)], F32)
            ar2_ap = ar2_in[:]
            ar2_perm = bass.AP(
                tensor=ar2_ap.tensor, offset=ar2_ap.offset,
                ap=[[4 * (D + 1), 64], [D + 1, 4], [64 * 4 * (D + 1), 2], [1, D + 1]])
            nc.sync.dma_start(out=ar2_perm,
                               in_=ocst.rearrange("p (a c) b -> p a c b", a=4))
            all_reduce(ar2_in, ar2_out)
            ocred = pB.tile([128, 4, D + 1], F32, tag="ocred")
            nc.sync.dma_start(out=ocred.rearrange("p a b -> p (a b)"), in_=ar2_out[:])

            # finish cross-attention + OS, stage-interleaved across heads
            oc2s, osfTs = {}, {}
            for hp in range(4):  # T1: cross-softmax normalize
                oc2 = pBh.tile([128, D], BF, tag="oc2")
                zc = pBh.tile([128, 1], F32, tag="zc")
                nc.vector.reciprocal(out=zc[:], in_=ocred[:, hp, D : D + 1])
                nc.vector.tensor_scalar_mul(out=oc2[:], in0=ocred[:, hp, 0:D],
                                            scalar1=zc[:])
                oc2s[hp] = oc2
            for hp, hh in heads:  # T2: transpose + residual add
                h = 2 * hp + hh
                src = oc2s[hp][0:64, :] if hh == 0 else oc2s[hp][64:128, :]
                idn = ident2[0:64, :] if hh == 0 else ident2[64:128, :]
                ocT_ps = psBs.tile([64, 64], BF, tag="small", bufs=4)
                nc.tensor.transpose(ocT_ps[:], src, idn)
                osfT = pBh.tile([64, 64], BF, tag="osfT")
                nc.vector.tensor_add(out=osfT[:], in0=ocT_ps[:],
                                     in1=osT_all[:, h, :])
                osfTs[(hp, hh)] = osfT
            osps = {}
            for hp, hh in heads:  # T3: project through Wo (pair shares a bank)
                h = 2 * hp + hh
                if hh == 0:
                    osps[hp] = psBs.tile([128, 512], F32, tag="osp", bufs=2,
                                         name=f"osp{hp}")
                nc.tensor.matmul(osps[hp][hh * 64 : hh * 64 + 64, 0:C],
                                 lhsT=osfTs.pop((hp, hh))[:],
                                 rhs=wo_sb[:, h, :], start=True, stop=True)
            for hp in range(4):  # T4: pack os_sb in one copy per pair
                osr = osps.pop(hp)
                if flags["bo"]:
                    nc.vector.tensor_add(out=os_sb[:, hp, :], in0=osr[:, 0:C],
                                         in1=bo_bc[:])
                else:
                    nc.vector.tensor_copy(out=os_sb[:, hp, :], in_=osr[:, 0:C])

        # ---------------- Phase C ----------------
        with contextlib.ExitStack() as phC:
            pC = phC.enter_context(tc.tile_pool(name="pC", bufs=2))
            psC = phC.enter_context(tc.tile_pool(name="psC", bufs=6, space="PSUM"))
            y_re = y.ap().rearrange("(s j p) c -> s p j c", j=8, p=128)
            for si in range(NSUP):
                stg = pC.tile([128, 8, C], BF, tag="stg")
                for j in range(8):
                    i = si * 8 + j
                    o_ps = psC.tile([128, C], F32, tag="o")
                    for cc in range(4):
                        nc.tensor.matmul(o_ps[:],
                                         lhsT=eT[:, cc, i * 128 : (i + 1) * 128],
                                         rhs=os_sb[:, cc, :],
                                         start=(cc == 0), stop=(cc == 3))
                    if i % 2 == 0:
                        nc.scalar.activation(out=stg[:, j, :], in_=o_ps[:],
                                             func=AF.Copy)
                    else:
                        nc.vector.tensor_copy(out=stg[:, j, :], in_=o_ps[:])
                nc.sync.dma_start(out=y_re[si], in_=stg[:])

    nc.compile()
    return nc


_CACHE: dict = {}


def _get_nc(n_cores: int, T: int, flags_key: tuple):
    key = (n_cores, T, flags_key)
    if key not in _CACHE:
        flags = dict(zip(("bias_s", "bqv", "bsp", "bck", "bfx", "bcv", "bo"), flags_key))
        _CACHE[key] = _build(n_cores, T, flags)
    return _CACHE[key]


def prep_inputs(inputs: dict, n_cores: int, T: int):
    """Host-side prep: transposes, weight folding, bf16 casts, per-core maps."""
    f32 = np.float32
    x = np.asarray(inputs["x"], f32)
    snt = np.asarray(inputs["sonata_features"], f32)
    temp = np.asarray(inputs["temperature"], f32).reshape(H)
    Wx, bx = np.asarray(inputs["Wx"], f32), np.asarray(inputs["bx"], f32)
    Wfx, bfx = np.asarray(inputs["Wfx"], f32), np.asarray(inputs["bfx"], f32)
    Wsl, bsl = np.asarray(inputs["Wslice"], f32), np.asarray(inputs["bslice"], f32)
    Wq, bq = np.asarray(inputs["Wq"], f32), np.asarray(inputs["bq"], f32)
    Wk, bk = np.asarray(inputs["Wk"], f32), np.asarray(inputs["bk"], f32)
    Wv, bv = np.asarray(inputs["Wv"], f32), np.asarray(inputs["bv"], f32)
    Wsp, bsp = np.asarray(inputs["Wsp"], f32), np.asarray(inputs["bsp"], f32)
    Wck, bck = np.asarray(inputs["Wck"], f32), np.asarray(inputs["bck"], f32)
    Wcv, bcv = np.asarray(inputs["Wcv"], f32), np.asarray(inputs["bcv"], f32)
    Wo, bo = np.asarray(inputs["Wo"], f32), np.asarray(inputs["bo"], f32)

    Wxs = np.zeros((C, HG), f32)
    bias_s = np.zeros((HG,), f32)
    for h in range(H):
        Wxs[:, h * G : (h + 1) * G] = (Wx[:, h * D : (h + 1) * D] @ Wsl) / temp[h]
        bias_s[h * G : (h + 1) * G] = (bx[h * D : (h + 1) * D] @ Wsl + bsl) / temp[h]
    flags = {
        "bias_s": bool(np.any(bias_s != 0)),
        "bqv": bool(np.any(bq != 0) or np.any(bk != 0) or np.any(bv != 0)),
        "bsp": bool(np.any(bsp != 0)),
        "bck": bool(np.any(bck != 0)),
        "bfx": bool(np.any(bfx != 0)),
        "bcv": bool(np.any(bcv != 0)),
        "bo": bool(np.any(bo != 0)),
    }
    w5 = np.stack([Wq * SCALE, Wk, Wv, Wck * SCALE, Wcv], axis=1)  # [D, 5, D]
    shared = {
        "wxs": np.ascontiguousarray(Wxs).astype(NPBF),
        "wfx": np.ascontiguousarray(Wfx).astype(NPBF),
        "wsp": np.ascontiguousarray(Wsp).astype(NPBF),
        "w5": np.ascontiguousarray(w5).astype(NPBF),
        "wo": np.ascontiguousarray(Wo).astype(NPBF),
        "bqv": np.ascontiguousarray(np.stack([bq * SCALE, bk, bv])),
        "bck": np.ascontiguousarray(bck * SCALE),
        "bsp": np.ascontiguousarray(bsp),
    }
    if flags["bias_s"]:
        shared["bias_s"] = bias_s
    if flags["bfx"]:
        shared["bfx"] = bfx
    if flags["bcv"]:
        shared["bcv"] = bcv
    if flags["bo"]:
        shared["bo"] = bo

    in_maps = []
    for c in range(n_cores):
        b, half = c // 2, c % 2
        xt_c = np.ascontiguousarray(x[b, half * T : (half + 1) * T, :].T).astype(NPBF)
        snt_c = np.ascontiguousarray(
            snt[b].T[:, half * MH : (half + 1) * MH]).astype(NPBF)
        in_maps.append({"xt": xt_c, "snt": snt_c, **shared})
    return in_maps, flags


def run(inputs: dict, n_cores: int = 8, T: int = N // 2, **spmd_kwargs):
    in_maps, flags = prep_inputs(inputs, n_cores, T)
    nc = _get_nc(n_cores, T, tuple(flags.values()))
    res = run_bass_kernel_spmd(nc, in_maps, core_ids=list(range(n_cores)),
                               **spmd_kwargs)
    out = np.zeros((B, N, C), np.float32)
    for c in range(n_cores):
        b, half = c // 2, c % 2
        out[b, half * T : (half + 1) * T, :] = np.asarray(
            res.results[c]["y"]).astype(np.float32)
    return out, res


def kernel(**inputs) -> np.ndarray:
    out, _ = run(inputs)
    return out


# revision 53
# speedup vs baseline: 1.0139x; 1.0018x over previous
"""Trainium2 Bass kernel for Enhanced Physics Attention with Sonata.

Contract: kernel(**inputs) takes FULL unsharded numpy inputs (as produced by
setup_inputs()) and returns the FULL [B, N, C] output. Internally shards
across 8 NeuronCores: core c handles batch c//2, token half c%2, and sonata
half c%2 (cross-attention partials). Two pairwise AllReduces: pooled slice
tokens after phase A, cross-attention numerators/denominators in phase B.

Math (fp32 PSUM accumulation, bf16 operands on the hot paths; rel err ~6e-3):
  Host folds Wslice+temperature into Wxs (s = x @ Wxs), SCALE into Wq/Wck,
  and casts x/weights to bf16.
  Phase A (token-major, lag-6 produce/consume software pipeline over
    pair-batched 128-token tiles): e = exp(s) [logits bounded ~3, no max
    needed], z = grouped sum (DVE), eh = e/z (gpsimd); pooling
    slice_token = eh^T @ (fx | 1) — the appended ones column yields
    slice_norm for free; eh^T (PE transpose + DVE 2x copy) stays fully
    resident in SBUF as bf16. Sonata projections + cross K/V are emitted
    interleaved mid-phase to fill engine slack.
  Phase B (stage-interleaved across heads so the in-order engine queues
    overlap the chains): slice self-attention (replicated) + sonata
    cross-attention over this core's sonata half; ones-column gives the
    softmax denominator, the pair AllReduce sums numerator+denominator
    over the full sonata. Both heads of a pair project through Wo into one
    full-bank PSUM tile (odd head via col tile_position) so os packs in
    one DVE copy.
  Phase C: out = eh_T.T @ (out_slice @ Wo), written back as bf16.

Self-contained: hardcodes all shapes; does not read sibling files.
"""

import contextlib
import sys

try:
    import concourse  # noqa: F401
except ImportError:
    sys.path.insert(0, "/opt/trn_rl_repo")

import ml_dtypes
import numpy as np

import concourse.bass as bass
import concourse.tile as tile
from concourse import bacc, mybir
from concourse.bass_utils import run_bass_kernel_spmd
from concourse.masks import make_identity

F32 = mybir.dt.float32
BF = mybir.dt.bfloat16
NPBF = np.dtype(ml_dtypes.bfloat16)
AF = mybir.ActivationFunctionType

# Problem shapes
B, N, C = 4, 16384, 256
H, D, G = 8, 64, 64
M, SD = 1024, 768
SCALE = D**-0.5
HG = H * G  # 512
HD = H * D  # 512
MH = M // 2  # sonata tokens per core (m-split across the pair)


def _bcast_ap(dram_ap: bass.AP, parts: int, n: int, offset: int = 0) -> bass.AP:
    """AP reading n contiguous DRAM floats, replicated across `parts` partitions."""
    return bass.AP(
        tensor=dram_ap.tensor,
        offset=dram_ap.offset + offset,
        ap=[[0, parts], [1, n]],
    )


def _fbcast(ap2d: bass.AP, rep: int) -> bass.AP:
    """[p, k] AP -> [p, k, rep] with step-0 innermost broadcast."""
    return bass.AP(tensor=ap2d.tensor, offset=ap2d.offset,
                   ap=[ap2d.ap[0], ap2d.ap[1], [0, rep]])


def _build(n_cores: int, T: int, flags: dict, no_collective: bool = False):
    """Build the per-core Bass module. T = tokens per core (multiple of 1024)."""
    assert T % 1024 == 0
    NSUP = T // 1024  # super-tiles (x loads)
    NT = T // 128  # 128-token tiles

    nc = bacc.Bacc(
        "TRN2", target_bir_lowering=False, debug=False, num_devices=n_cores
    )

    # ---- DRAM I/O ----
    xt = nc.dram_tensor("xt", [C, T], BF, kind="ExternalInput")  # x slice, transposed
    snt = nc.dram_tensor("snt", [SD, MH], BF, kind="ExternalInput")  # sonata half, T
    wxs = nc.dram_tensor("wxs", [C, HG], BF, kind="ExternalInput")
    wfx = nc.dram_tensor("wfx", [C, HD], BF, kind="ExternalInput")
    wsp = nc.dram_tensor("wsp", [SD, HD], BF, kind="ExternalInput")
    w5 = nc.dram_tensor("w5", [D, 5, D], BF, kind="ExternalInput")  # q,k,v,ck,cv
    wo = nc.dram_tensor("wo", [HD, C], BF, kind="ExternalInput")
    bqv = nc.dram_tensor("bqv", [3, D], F32, kind="ExternalInput")  # bq,bk,bv rows
    bck = nc.dram_tensor("bck", [D], F32, kind="ExternalInput")
    bsp = nc.dram_tensor("bsp", [HD], F32, kind="ExternalInput")
    if flags["bias_s"]:
        bias_s = nc.dram_tensor("bias_s", [HG], F32, kind="ExternalInput")
    if flags["bfx"]:
        bfx = nc.dram_tensor("bfx", [HD], F32, kind="ExternalInput")
    if flags["bcv"]:
        bcv = nc.dram_tensor("bcv", [D], F32, kind="ExternalInput")
    if flags["bo"]:
        bo = nc.dram_tensor("bo", [C], F32, kind="ExternalInput")
    y = nc.dram_tensor("y", [T, C], BF, kind="ExternalOutput")

    groups = [[2 * i, 2 * i + 1] for i in range(n_cores // 2)]

    def all_reduce(ar_in, ar_out):
        if no_collective:
            nc.gpsimd.dma_start(out=ar_out[:], in_=ar_in[:])
        else:
            nc.gpsimd.collective_compute(
                "AllReduce", mybir.AluOpType.add, replica_groups=groups,
                ins=[ar_in.opt()], outs=[ar_out.opt()])

    with tile.TileContext(nc) as tc, contextlib.ExitStack() as top:
        singles = top.enter_context(tc.tile_pool(name="singles", bufs=1))
        dram = top.enter_context(tc.tile_pool(name="dram", bufs=1, space="DRAM"))

        # ---- resident weights / inputs ----
        # Emission order matters: the in-order DMA queues must deliver wxs/wfx
        # and the first x super-tile before anything else so PE starts ASAP.
        wxs_sb = singles.tile([128, 2, HG], BF)
        wfx_sb = singles.tile([128, 2, HD], BF)
        wxs_re = wxs.ap().rearrange("(k p) n -> p k n", p=128)
        wfx_re = wfx.ap().rearrange("(k p) n -> p k n", p=128)
        # k0 chunks first so the very first matmuls can start sooner
        nc.sync.dma_start(out=wxs_sb[:, 0, :], in_=wxs_re[:, 0, :])
        ident = singles.tile([128, 128], BF)
        warm = singles.tile([1, 1], F32)
        nc.vector.memset(warm[:], 0.0)
        nc.scalar.activation(out=warm[:], in_=warm[:], func=AF.Exp)
        make_identity(nc, ident[:])
        # stacked eye(64)s: lets transposes consume partition-64-based [64,64]
        # sources directly (identity operand must share the source's base)
        ident2 = singles.tile([128, 64], BF)
        make_identity(nc, ident2[0:64, :])
        make_identity(nc, ident2[64:128, :])

        # tiles for deferred loads (DMAs emitted after the first super-tile)
        snt_sb = singles.tile([128, 6, MH], BF)
        wsp_sb = singles.tile([128, 6, HD], BF)
        w64 = singles.tile([64, 5, D], BF)  # wq,wk,wv,wck,wcv
        wo_sb = singles.tile([64, H, C], BF)

        def load_phaseB_weights():
            nc.sync.dma_start(out=snt_sb[:],
                              in_=snt.ap().rearrange("(k p) m -> p k m", p=128))
            nc.sync.dma_start(out=wsp_sb[:],
                              in_=wsp.ap().rearrange("(k p) n -> p k n", p=128))
            nc.sync.dma_start(out=w64[:], in_=w5.ap())
            nc.sync.dma_start(out=wo_sb[:],
                              in_=wo.ap().rearrange("(h d) c -> d h c", d=64))

        # eh^T: chunk c holds heads 2c,2c+1 stacked on partitions; fully
        # resident in SBUF as bf16 (4 * T * 2 bytes per partition).
        eT = singles.tile([128, 4, NT * 128], BF)
        # OS (slice-token outputs @ Wo), chunk-packed like eT
        os_sb = singles.tile([128, 4, C], BF)

        # (fx | 1) pooling rhs: manual 3-buffer rotation so the ones column
        # is preset exactly once per buffer.
        fxe_bufs = [singles.tile([128, 2, H, D + 1], BF, name=f"fxe{k}")
                    for k in range(5)]
        for fb in fxe_bufs:
            nc.vector.memset(fb[:, :, :, D], 1.0)

        # small per-partition bias columns
        bqv_c = singles.tile([64, 3], F32)
        nc.sync.dma_start(out=bqv_c[:], in_=bqv.ap().rearrange("q d -> d q"))
        bck_c = singles.tile([64, 1], F32)
        nc.sync.dma_start(out=bck_c[:], in_=bck.ap().rearrange("(d o) -> d o", o=1))
        bsp_c = singles.tile([128, 4], F32)
        nc.sync.dma_start(out=bsp_c[:], in_=bsp.ap().rearrange("(k p) -> p k", p=128))

        if flags["bias_s"]:
            bias_s_bc = singles.tile([128, HG], F32)
            nc.sync.dma_start(out=bias_s_bc[:], in_=_bcast_ap(bias_s.ap(), 128, HG))
        if flags["bcv"]:
            bcv_bc = singles.tile([128, D], F32)
            nc.sync.dma_start(out=bcv_bc[:], in_=_bcast_ap(bcv.ap(), 128, D))
        if flags["bo"]:
            # bo/H replicated on all 128 partitions (see phase B T4: each of
            # the H unpool rows carries bo/H, and per-head eh sums to 1)
            bo_bc = singles.tile([128, C], F32)
            nc.sync.dma_start(out=bo_bc[:], in_=_bcast_ap(bo.ap(), 128, C))
            nc.vector.tensor_scalar_mul(out=bo_bc[:], in0=bo_bc[:],
                                        scalar1=1.0 / H)
        if flags["bfx"]:
            bfx_bc = singles.tile([128, D], F32)

        # sonata-side SBUF tiles (produced during phase A, consumed in phase B)
        sfT = singles.tile([64, H, MH], BF)  # sf^T [d, h, m] head-major
        ksT_all = singles.tile([64, H, MH], BF)
        NMC = MH // 128  # m-chunks on this core
        vse_all = singles.tile([128, H, NMC, D + 1], BF)
        nc.vector.memset(vse_all[:, :, :, D : D + 1], 1.0)

        # ---------------- Phase A ----------------
        LAG = 8  # tiles of software-pipeline lag (even: produce/consume pair tiles)
        with contextlib.ExitStack() as phA:
            pA = phA.enter_context(tc.tile_pool(name="pA", bufs=5))
            pAx = phA.enter_context(tc.tile_pool(name="pAx", bufs=8))
            psA = phA.enter_context(tc.tile_pool(name="psA", bufs=2, space="PSUM"))
            psAcc = phA.enter_context(tc.tile_pool(name="psAcc", bufs=1, space="PSUM"))
            psH = phA.enter_context(tc.tile_pool(name="psH", bufs=1, space="PSUM"))

            # All 8 head accumulators live on partitions 0-63 across two full
            # PSUM banks (2KB-exact per-partition stride). Only the first
            # matmul touching each bank (h==0 / h==4 at tile 0) uses
            # start=True: the bank-wide has_written clear makes heads 1-3 /
            # 5-7 of tile 0 overwrite, and all later tiles accumulate.
            pool_acc = psAcc.tile([64, H, 128], F32, name="poolacc", tag="acc")

            # --- hoisted phase-B producers: sonata projection + cross K/V ---
            # Emitted interleaved into phase A to fill engine slack.
            def unit_sf(hd):  # hd chunk = heads 2hd, 2hd+1
                ps = psH.tile([128, MH], F32, tag="hps")
                for sd in range(6):
                    nc.tensor.matmul(
                        ps[:], lhsT=wsp_sb[:, sd, hd * 128 : (hd + 1) * 128],
                        rhs=snt_sb[:, sd, :], start=(sd == 0), stop=(sd == 5))
                up = pA.tile([128, MH], BF, tag="sfup")
                if flags["bsp"]:
                    nc.scalar.activation(out=sfT[:, 2 * hd, :], in_=ps[0:64, :],
                                         func=AF.Identity,
                                         bias=bsp_c[0:64, hd : hd + 1])
                    nc.scalar.activation(out=up[64:128, :], in_=ps[64:128, :],
                                         func=AF.Identity,
                                         bias=bsp_c[64:128, hd : hd + 1])
                else:
                    nc.vector.tensor_copy(out=sfT[:, 2 * hd, :], in_=ps[0:64, :])
                    nc.vector.tensor_copy(out=up[64:128, :], in_=ps[64:128, :])
                nc.gpsimd.dma_start(out=sfT[:, 2 * hd + 1, :], in_=up[64:128, :])

            def unit_kv(h):
                kp = psH.tile([64, MH], F32, tag="hps")
                nc.tensor.matmul(kp[:], lhsT=w64[:, 3, :], rhs=sfT[:, h, :],
                                 start=True, stop=True)
                nc.scalar.activation(out=ksT_all[:, h, :], in_=kp[:],
                                     func=AF.Identity, bias=bck_c[:])
                vp4 = psH.tile([128, NMC, 128], F32, tag="hps")
                for mo in range(NMC):
                    nc.tensor.matmul(vp4[:, mo, 0:64],
                                     lhsT=sfT[:, h, mo * 128 : (mo + 1) * 128],
                                     rhs=w64[:, 4, :], start=True, stop=True)
                if flags["bcv"]:
                    bcv4 = bass.AP(tensor=bcv_bc[:].tensor, offset=bcv_bc[:].offset,
                                   ap=[bcv_bc[:].ap[0], [0, NMC], [1, D]])
                    nc.vector.tensor_add(out=vse_all[:, h, :, 0:D],
                                         in0=vp4[:, :, 0:64], in1=bcv4)
                else:
                    nc.vector.tensor_copy(out=vse_all[:, h, :, 0:D],
                                          in_=vp4[:, :, 0:64])

            units = [lambda hd=hd: unit_sf(hd) for hd in range(4)]
            units += [lambda h=h: unit_kv(h) for h in range(H)]
            # spread units over mid-phase tiles (needs weights from si==0 DMAs)
            if NT >= 52:
                unit_at = {16 + (3 * k) // 2 * 2: k for k in range(len(units))}
            else:
                unit_at = {}
            units_done = [False] * len(units)

            handles = {}

            def produce2(i0, xt_sb, j0):
                # tiles i0, i0+1 batched: one ACT/DVE/Pool op per stage pair
                # amortizes the fixed memory-access cost of each instruction.
                s_ps = psA.tile([128, 2, HG], F32, tag="s", bufs=1)
                fx_ps = psA.tile([128, 2, HD], F32, tag="fx", bufs=1)
                for t in range(2):
                    tok = slice((j0 + t) * 128, (j0 + t + 1) * 128)
                    nc.tensor.matmul(s_ps[:, t, :], lhsT=xt_sb[:, 0, tok],
                                     rhs=wxs_sb[:, 0, :], start=True, stop=False)
                    nc.tensor.matmul(s_ps[:, t, :], lhsT=xt_sb[:, 1, tok],
                                     rhs=wxs_sb[:, 1, :], start=False, stop=True)
                for t in range(2):
                    tok = slice((j0 + t) * 128, (j0 + t + 1) * 128)
                    nc.tensor.matmul(fx_ps[:, t, :], lhsT=xt_sb[:, 0, tok],
                                     rhs=wfx_sb[:, 0, :], start=True, stop=False)
                    nc.tensor.matmul(fx_ps[:, t, :], lhsT=xt_sb[:, 1, tok],
                                     rhs=wfx_sb[:, 1, :], start=False, stop=True)

                e2 = pA.tile([128, 2, H, G], BF, tag="e")
                eh2 = pA.tile([128, 2, H, G], BF, tag="eh", bufs=11)
                z2 = pA.tile([128, 2, H], F32, tag="z")
                fxe2 = fxe_bufs[(i0 // 2) % len(fxe_bufs)]
                # Steady state: pair-batched ops, normalize on gpsimd (ACT/DVE
                # are the pacers). Pipeline-fill and drain batches: per-tile
                # ops on the shorter DVE path — their chain latency is exposed.
                split = i0 == NT - 2
                dve_tt = i0 < 2 or i0 >= NT - 2
                parts = ([slice(t, t + 1) for t in range(2)]
                         if split else [slice(0, 2)])
                for ts in parts:
                    nts = ts.stop - ts.start
                    ef = e2[:, ts, :, :].rearrange("p t a b -> p (t a b)")
                    if flags["bias_s"]:
                        bias2 = bass.AP(tensor=bias_s_bc[:].tensor,
                                        offset=bias_s_bc[:].offset,
                                        ap=[bias_s_bc[:].ap[0], [0, nts], [1, HG]])
                        s_sb = pA.tile([128, 2, HG], F32, tag="ssb")
                        nc.vector.tensor_add(out=s_sb[:, ts, :],
                                             in0=s_ps[:, ts, :], in1=bias2)
                        nc.scalar.activation(out=ef, in_=s_sb[:, ts, :].rearrange(
                            "p t n -> p (t n)"), func=AF.Exp)
                    else:
                        nc.scalar.activation(
                            out=ef, in_=s_ps[:, ts, :].rearrange("p t n -> p (t n)"),
                            func=AF.Exp)
                for ts in parts:
                    nts = ts.stop - ts.start
                    nc.vector.reduce_sum(out=z2[:, ts, :], in_=e2[:, ts, :, :],
                                         axis=mybir.AxisListType.X)
                    zsl = z2[:, ts, :]
                    nc.vector.reciprocal(
                        out=zsl.rearrange("p t a -> p (t a)"),
                        in_=zsl.rearrange("p t a -> p (t a)"))
                    zrb = bass.AP(tensor=zsl.tensor, offset=zsl.offset,
                                  ap=[zsl.ap[0], [H, nts], [1, H], [0, G]])
                    if dve_tt:
                        nc.vector.tensor_tensor(out=eh2[:, ts, :, :],
                                                in0=e2[:, ts, :, :], in1=zrb,
                                                op=mybir.AluOpType.mult)
                    else:
                        nc.gpsimd.tensor_tensor(out=eh2[:, ts, :, :],
                                                in0=e2[:, ts, :, :], in1=zrb,
                                                op=mybir.AluOpType.mult)
                # pooling rhs = (fx | 1), ones columns preset per buffer
                nc.scalar.activation(
                    out=fxe2[:, :, :, 0:D],
                    in_=fx_ps.rearrange("p t (a b) -> p t a b", a=H),
                    func=AF.Copy)
                handles[i0] = (eh2, fxe2)

            deferred = []

            def consume2(i0, defer=False):
                eh2, fxe2 = handles.pop(i0)
                for t in range(2):
                    i = i0 + t
                    for h in range(H):
                        nc.tensor.matmul(
                            pool_acc[0:64, h, 0 : D + 1],
                            lhsT=eh2[:, t, h, :], rhs=fxe2[:, t, h, :],
                            start=(i == 0 and h % 4 == 0), stop=(i == NT - 1),
                            skip_group_check=True)
                if defer:
                    # eh^T production does not gate the AllReduce — emit it
                    # after the AR staging to fill the collective round trip.
                    deferred.append((i0, eh2))
                    return
                transpose_out(i0, eh2)

            def transpose_out(i0, eh2):
                etp = psA.tile([128, 2, 4, 128], BF, tag="etp", bufs=1)
                ehf = eh2.rearrange("p t a b -> p t (a b)")
                for t in range(2):
                    for cc in range(4):
                        nc.tensor.transpose(etp[:, t, cc, :],
                                            ehf[:, t, cc * 128 : (cc + 1) * 128],
                                            ident[:])
                nc.vector.tensor_copy(
                    out=eT[:, :, i0 * 128 : (i0 + 2) * 128].rearrange(
                        "p c (t k) -> p c t k", t=2),
                    in_=etp.rearrange("p t c k -> p c t k"))

            xt_re = xt.ap().rearrange("(k p) n -> p k n", p=128)
            # Prefetch every x super-tile up front: the SP queue then serves
            # the eh^T transposes without ever blocking an x load behind them.
            xt_tiles = []
            for si in range(NSUP):
                xt_sb = pAx.tile([128, 2, 1024], BF, tag="xt", name=f"xt{si}")
                sl = slice(si * 1024, (si + 1) * 1024)
                if si == 0:
                    # finest-grained first loads: the tile-0 matmuls only need
                    # the first 256 tokens of each k-chunk
                    nc.sync.dma_start(out=xt_sb[:, 0, 0:256], in_=xt_re[:, 0, 0:256])
                    nc.sync.dma_start(out=wfx_sb[:, 0, :], in_=wfx_re[:, 0, :])
                    nc.sync.dma_start(out=xt_sb[:, 0, 256:1024],
                                      in_=xt_re[:, 0, slice(256, 1024)])
                    nc.sync.dma_start(out=wxs_sb[:, 1, :], in_=wxs_re[:, 1, :])
                    nc.sync.dma_start(out=xt_sb[:, 1, 0:256], in_=xt_re[:, 1, 0:256])
                    nc.sync.dma_start(out=wfx_sb[:, 1, :], in_=wfx_re[:, 1, :])
                    nc.sync.dma_start(out=xt_sb[:, 1, 256:1024],
                                      in_=xt_re[:, 1, slice(256, 1024)])
                else:
                    nc.sync.dma_start(out=xt_sb[:], in_=xt_re[:, :, sl])
                xt_tiles.append(xt_sb)
            for si in range(NSUP):
                xt_sb = xt_tiles[si]
                for j in range(0, 8, 2):
                    i = si * 8 + j
                    produce2(i, xt_sb, j)
                    if i >= LAG:
                        consume2(i - LAG)
                    if i in unit_at:
                        k = unit_at.pop(i)
                        units[k]()
                        units_done[k] = True
                if si == 0:
                    load_phaseB_weights()
            DEFER = 0  # deferred eh^T emission disabled (HW-divergent)
            for i in range(NT - LAG, NT, 2):
                consume2(i, defer=(i >= NT - DEFER))
            for k, u in enumerate(units):  # emit any units not yet scheduled
                if not units_done[k]:
                    u()

            # pooled partials -> AllReduce across the pair. The accumulator is
            # [64p, 8h, 65]; the AR payload (and phase B) use the pair layout
            # [128p = 64p x 2(h odd/even), 4 pairs, 65] — the DRAM staging DMA
            # applies the permutation (DRAM side is fully linear).
            pool_sb = pA.tile([64, H, D + 1], F32, tag="poolsb")
            nc.scalar.activation(out=pool_sb[:], in_=pool_acc[:, :, 0 : D + 1],
                                 func=AF.Copy)
            ar_in = dram.tile([128, 4 * (D + 1)], F32)
            ar_out = dram.tile([128, 4 * (D + 1)], F32)
            ar_in_ap = ar_in[:]
            ar_in_perm = bass.AP(
                tensor=ar_in_ap.tensor, offset=ar_in_ap.offset,
                # lockstep with pool_sb [64p][4 hp][2 hh][65]: dram row
                # p + 64*hh, column hp*65 + b
                ap=[[4 * (D + 1), 64], [D + 1, 4], [64 * 4 * (D + 1), 2], [1, D + 1]])
            nc.sync.dma_start(
                out=ar_in_perm,
                in_=pool_sb.rearrange("p (a c) b -> p a c b", a=4))
            all_reduce(ar_in, ar_out)

        # ---------------- Phase B ----------------
        with contextlib.ExitStack() as phB:
            pBw = phB.enter_context(tc.tile_pool(name="pBw", bufs=1))
            pB = phB.enter_context(tc.tile_pool(name="pB", bufs=2))
            pBh = phB.enter_context(tc.tile_pool(name="pBh", bufs=8))
            psB = phB.enter_context(tc.tile_pool(name="psB", bufs=2, space="PSUM"))
            psBs = phB.enter_context(tc.tile_pool(name="psBs", bufs=1, space="PSUM"))

            pool_red = pB.tile([128, 4, D + 1], F32, tag="poolred")
            nc.sync.dma_start(out=pool_red.rearrange("p a b -> p (a b)"), in_=ar_out[:])

            # Stage-interleaved emission: each stage is emitted for all heads
            # before the next stage, so the in-order engine queues overlap the
            # independent per-head chains instead of running them serially.
            ocst = pBw.tile([64, H, D + 1], F32)  # cross-attn partials, h-major
            osT_all = pBw.tile([64, H, 64], BF)  # self-attn out^T per head
            heads = [(hp, hh) for hp in range(4) for hh in range(2)]
            st2s, stTs, qkvTs = {}, {}, {}
            d_a, d_ea, d_za, d_pa, d_vsb, d_pat, d_ecT, d_oc = ({} for _ in range(8))

            for hp in range(4):  # S1: slice-token normalize
                pr = pool_red[:, hp, :]  # [128, 65]: heads 2hp (low), 2hp+1 (hi)
                nrm = pBh.tile([128, 1], F32, tag="nrm")
                nc.vector.tensor_scalar_add(out=nrm[:], in0=pr[:, D : D + 1],
                                            scalar1=1e-5)
                nc.vector.reciprocal(out=nrm[:], in_=nrm[:])
                st2 = pBh.tile([128, D], BF, tag="st2")
                if flags["bfx"]:
                    for hh in range(2):
                        h = 2 * hp + hh
                        sl = slice(hh * 64, hh * 64 + 64)
                        nc.sync.dma_start(out=bfx_bc[sl, :],
                                          in_=_bcast_ap(bfx.ap(), 64, D, offset=h * D))
                    tmpb = pBh.tile([128, D], F32, tag="tmpb")
                    nc.vector.tensor_scalar_mul(out=tmpb[:], in0=bfx_bc[:],
                                                scalar1=pr[:, D : D + 1])
                    nc.vector.tensor_add(out=tmpb[:], in0=tmpb[:], in1=pr[:, 0:D])
                    nc.vector.tensor_scalar_mul(out=st2[:], in0=tmpb[:], scalar1=nrm[:])
                else:
                    nc.vector.tensor_scalar_mul(out=st2[:], in0=pr[:, 0:D],
                                                scalar1=nrm[:])
                st2s[hp] = st2
            for hp in range(4):  # S2: transpose slice tokens
                stT_ps = psBs.tile([64, 128], BF, tag="small", bufs=4)
                nc.tensor.transpose(stT_ps[:], st2s[hp][:], ident[:])
                stT = pBh.tile([64, 128], BF, tag="stT")
                nc.vector.tensor_copy(out=stT[:], in_=stT_ps[:])
                stTs[hp] = stT
            for hp in range(4):  # S3: q/k/v projections (batched per pair)
                qkvTs[hp] = pBh.tile([64, 3, 128], BF, tag="qkvT",
                                     name=f"qkvT{hp}")
                qp = psBs.tile([64, 3, 128], F32, tag="small", bufs=4,
                               name=f"qp{hp}")
                for idx in range(3):
                    nc.tensor.matmul(qp[:, idx, :], lhsT=w64[:, idx, :],
                                     rhs=stTs[hp][:], start=True, stop=True)
                if flags["bqv"]:
                    for idx in range(3):
                        nc.scalar.activation(out=qkvTs[hp][:, idx, :],
                                             in_=qp[:, idx, :], func=AF.Identity,
                                             bias=bqv_c[:, idx : idx + 1])
                elif hp % 2 == 0:
                    nc.vector.tensor_copy(out=qkvTs[hp][:], in_=qp[:])
                else:
                    nc.scalar.activation(out=qkvTs[hp].rearrange("p a b -> p (a b)"),
                                         in_=qp.rearrange("p a b -> p (a b)"),
                                         func=AF.Copy)
            for hp, hh in heads:  # S4: self-attention logits
                hs = slice(hh * 64, hh * 64 + 64)
                a_ps = psBs.tile([64, 64], F32, tag="small", bufs=4)
                nc.tensor.matmul(a_ps[:], lhsT=qkvTs[hp][:, 0, hs],
                                 rhs=qkvTs[hp][:, 1, hs], start=True, stop=True)
                d_a[(hp, hh)] = a_ps
            for hp, hh in heads:  # S5: softmax exp, then row sums on DVE
                ea = pBh.tile([64, 64], F32, tag="ea")
                nc.scalar.activation(out=ea[:], in_=d_a.pop((hp, hh))[:],
                                     func=AF.Exp)
                za = pBh.tile([64, 1], F32, tag="za")
                nc.vector.reduce_sum(out=za[:], in_=ea[:],
                                     axis=mybir.AxisListType.X)
                d_ea[(hp, hh)], d_za[(hp, hh)] = ea, za
            for hp, hh in heads:  # S6: normalize attention
                za = d_za.pop((hp, hh))
                nc.vector.reciprocal(out=za[:], in_=za[:])
                pa = pBh.tile([64, 64], BF, tag="pa")
                nc.vector.tensor_scalar_mul(out=pa[:], in0=d_ea.pop((hp, hh))[:],
                                            scalar1=za[:])
                d_pa[(hp, hh)] = pa
            for hp, hh in heads:  # S7: transpose v and attention (batched)
                hs = slice(hh * 64, hh * 64 + 64)
                vp_ps = psBs.tile([64, 2, 64], BF, tag="small", bufs=4)
                nc.tensor.transpose(vp_ps[:, 0, :], qkvTs[hp][:, 2, hs],
                                    ident[0:64, 0:64])
                nc.tensor.transpose(vp_ps[:, 1, :], d_pa.pop((hp, hh))[:],
                                    ident[0:64, 0:64])
                vpat = pBh.tile([64, 2, 64], BF, tag="vpat")
                nc.vector.tensor_copy(out=vpat[:], in_=vp_ps[:])
                d_vsb[(hp, hh)] = vpat
            for hp, hh in heads:  # S8: self-attention output
                h = 2 * hp + hh
                vpat = d_vsb.pop((hp, hh))
                osf_ps = psBs.tile([64, 64], F32, tag="small", bufs=4)
                nc.tensor.matmul(osf_ps[:], lhsT=vpat[:, 0, :],
                                 rhs=vpat[:, 1, :], start=True, stop=True)
                nc.vector.tensor_copy(out=osT_all[:, h, :], in_=osf_ps[:])
            for hp, hh in heads:  # S9: cross-attention logits
                h = 2 * hp + hh
                ct_ps = psB.tile([128, NMC, 64], F32, tag="ct", bufs=2)
                for mo in range(NMC):
                    nc.tensor.matmul(ct_ps[:, mo, :],
                                     lhsT=ksT_all[:, h, mo * 128 : (mo + 1) * 128],
                                     rhs=osT_all[:, h, :], start=True, stop=True)
                d_a[(hp, hh)] = ct_ps
            for hp, hh in heads:  # S10: cross-attention exp
                ecT = pBh.tile([128, NMC, 64], BF, tag="ecT")
                ct_ps = d_a.pop((hp, hh))
                nc.scalar.activation(out=ecT.rearrange("p a b -> p (a b)"),
                                     in_=ct_ps.rearrange("p a b -> p (a b)"),
                                     func=AF.Exp)
                d_ecT[(hp, hh)] = ecT
            for hp, hh in heads:  # S11: cross numerator/denominator partials
                h = 2 * hp + hh
                ecT = d_ecT.pop((hp, hh))
                oc_ps = psBs.tile([64, 128], F32, tag="small", bufs=4)
                for mo in range(NMC):
                    nc.tensor.matmul(oc_ps[:, 0 : D + 1], lhsT=ecT[:, mo, :],
                                     rhs=vse_all[:, h, mo, :],
                                     start=(mo == 0), stop=(mo == NMC - 1))
                d_oc[(hp, hh)] = oc_ps
            for hp, hh in heads:  # S12: pack for the pair AllReduce
                h = 2 * hp + hh
                nc.vector.tensor_copy(out=ocst[0:64, h, :],
                                      in_=d_oc.pop((hp, hh))[:, 0 : D + 1])

            ar2_in = dram.tile([128, 4 * (D + 1)], F32)
            ar2_out = dram.tile([128, 4 * (D + 1)], F32)
            ar2_ap = ar2_in[:]
            ar2_perm = bass.AP(
                tensor=ar2_ap.tensor, offset=ar2_ap.offset,
                ap=[[4 * (D + 1), 64], [D + 1, 4], [64 * 4 * (D + 1), 2], [1, D + 1]])
            nc.sync.dma_start(out=ar2_perm,
                               in_=ocst.rearrange("p (a c) b -> p a c b", a=4))
            all_reduce(ar2_in, ar2_out)
            ocred = pB.tile([128, 4, D + 1], F32, tag="ocred")
            nc.sync.dma_start(out=ocred.rearrange("p a b -> p (a b)"), in_=ar2_out[:])

            # finish cross-attention + OS, stage-interleaved across heads
            oc2s, osfTs = {}, {}
            for hp in range(4):  # T1: cross-softmax normalize
                oc2 = pBh.tile([128, D], BF, tag="oc2")
                zc = pBh.tile([128, 1], F32, tag="zc")
                nc.vector.reciprocal(out=zc[:], in_=ocred[:, hp, D : D + 1])
                nc.vector.tensor_scalar_mul(out=oc2[:], in0=ocred[:, hp, 0:D],
                                            scalar1=zc[:])
                oc2s[hp] = oc2
            for hp, hh in heads:  # T2: transpose + residual add
                h = 2 * hp + hh
                src = oc2s[hp][0:64, :] if hh == 0 else oc2s[hp][64:128, :]
                idn = ident2[0:64, :] if hh == 0 else ident2[64:128, :]
                ocT_ps = psBs.tile([64, 64], BF, tag="small", bufs=4)
                nc.tensor.transpose(ocT_ps[:], src, idn)
                osfT = pBh.tile([64, 64], BF, tag="osfT")
                nc.vector.tensor_add(out=osfT[:], in0=ocT_ps[:],
                                     in1=osT_all[:, h, :])
                osfTs[(hp, hh)] = osfT
            osps = {}
            for hp, hh in heads:  # T3: project through Wo (pair shares a bank)
                h = 2 * hp + hh
                if hh == 0:
                    osps[hp] = psBs.tile([128, 512], F32, tag="osp", bufs=2,
                                         name=f"osp{hp}")
                nc.tensor.matmul(osps[hp][hh * 64 : hh * 64 + 64, 0:C],
                                 lhsT=osfTs.pop((hp, hh))[:],
                                 rhs=wo_sb[:, h, :], start=True, stop=True)
            for hp in range(4):  # T4: pack os_sb in one copy per pair
                osr = osps.pop(hp)
                if flags["bo"]:
                    nc.vector.tensor_add(out=os_sb[:, hp, :], in0=osr[:, 0:C],
                                         in1=bo_bc[:])
                else:
                    nc.vector.tensor_copy(out=os_sb[:, hp, :], in_=osr[:, 0:C])

        # ---------------- Phase C ----------------
        with contextlib.ExitStack() as phC:
            pC = phC.enter_context(tc.tile_pool(name="pC", bufs=2))
            psC = phC.enter_context(tc.tile_pool(name="psC", bufs=6, space="PSUM"))
            y_re = y.ap().rearrange("(s j p) c -> s p j c", j=8, p=128)
            for si in range(NSUP):
                stg = pC.tile([128, 8, C], BF, tag="stg")
                for j in range(8):
                    i = si * 8 + j
                    o_ps = psC.tile([128, C], F32, tag="o")
                    for cc in range(4):
                        nc.tensor.matmul(o_ps[:],
                                         lhsT=eT[:, cc, i * 128 : (i + 1) * 128],
                                         rhs=os_sb[:, cc, :],
                                         start=(cc == 0), stop=(cc == 3))
                    if i % 2 == 0:
                        nc.scalar.activation(out=stg[:, j, :], in_=o_ps[:],
                                             func=AF.Copy)
                    else:
                        nc.vector.tensor_copy(out=stg[:, j, :], in_=o_ps[:])
                nc.sync.dma_start(out=y_re[si], in_=stg[:])

    nc.compile()
    return nc


_CACHE: dict = {}


def _get_nc(n_cores: int, T: int, flags_key: tuple):
    key = (n_cores, T, flags_key)
    if key not in _CACHE:
        flags = dict(zip(("bias_s", "bqv", "bsp", "bck", "bfx", "bcv", "bo"), flags_key))
        _CACHE[key] = _build(n_cores, T, flags)
    return _CACHE[key]


def prep_inputs(inputs: dict, n_cores: int, T: int):
    """Host-side prep: transposes, weight folding, bf16 casts, per-core maps."""
    f32 = np.float32
    x = np.asarray(inputs["x"], f32)
    snt = np.asarray(inputs["sonata_features"], f32)
    temp = np.asarray(inputs["temperature"], f32).reshape(H)
    Wx, bx = np.asarray(inputs["Wx"], f32), np.asarray(inputs["bx"], f32)
    Wfx, bfx = np.asarray(inputs["Wfx"], f32), np.asarray(inputs["bfx"], f32)
    Wsl, bsl = np.asarray(inputs["Wslice"], f32), np.asarray(inputs["bslice"], f32)
    Wq, bq = np.asarray(inputs["Wq"], f32), np.asarray(inputs["bq"], f32)
    Wk, bk = np.asarray(inputs["Wk"], f32), np.asarray(inputs["bk"], f32)
    Wv, bv = np.asarray(inputs["Wv"], f32), np.asarray(inputs["bv"], f32)
    Wsp, bsp = np.asarray(inputs["Wsp"], f32), np.asarray(inputs["bsp"], f32)
    Wck, bck = np.asarray(inputs["Wck"], f32), np.asarray(inputs["bck"], f32)
    Wcv, bcv = np.asarray(inputs["Wcv"], f32), np.asarray(inputs["bcv"], f32)
    Wo, bo = np.asarray(inputs["Wo"], f32), np.asarray(inputs["bo"], f32)

    Wxs = np.zeros((C, HG), f32)
    bias_s = np.zeros((HG,), f32)
    for h in range(H):
        Wxs[:, h * G : (h + 1) * G] = (Wx[:, h * D : (h + 1) * D] @ Wsl) / temp[h]
        bias_s[h * G : (h + 1) * G] = (bx[h * D : (h + 1) * D] @ Wsl + bsl) / temp[h]
    flags = {
        "bias_s": bool(np.any(bias_s != 0)),
        "bqv": bool(np.any(bq != 0) or np.any(bk != 0) or np.any(bv != 0)),
        "bsp": bool(np.any(bsp != 0)),
        "bck": bool(np.any(bck != 0)),
        "bfx": bool(np.any(bfx != 0)),
        "bcv": bool(np.any(bcv != 0)),
        "bo": bool(np.any(bo != 0)),
    }
    w5 = np.stack([Wq * SCALE, Wk, Wv, Wck * SCALE, Wcv], axis=1)  # [D, 5, D]
    shared = {
        "wxs": np.ascontiguousarray(Wxs).astype(NPBF),
        "wfx": np.ascontiguousarray(Wfx).astype(NPBF),
        "wsp": np.ascontiguousarray(Wsp).astype(NPBF),
        "w5": np.ascontiguousarray(w5).astype(NPBF),
        "wo": np.ascontiguousarray(Wo).astype(NPBF),
        "bqv": np.ascontiguousarray(np.stack([bq * SCALE, bk, bv])),
        "bck": np.ascontiguousarray(bck * SCALE),
        "bsp": np.ascontiguousarray(bsp),
    }
    if flags["bias_s"]:
        shared["bias_s"] = bias_s
    if flags["bfx"]:
        shared["bfx"] = bfx
    if flags["bcv"]:
        shared["bcv"] = bcv
    if flags["bo"]:
        shared["bo"] = bo

    in_maps = []
    for c in range(n_cores):
        b, half = c // 2, c % 2
        xt_c = np.ascontiguousarray(x[b, half * T : (half + 1) * T, :].T).astype(NPBF)
        snt_c = np.ascontiguousarray(
            snt[b].T[:, half * MH : (half + 1) * MH]).astype(NPBF)
        in_maps.append({"xt": xt_c, "snt": snt_c, **shared})
    return in_maps, flags


def run(inputs: dict, n_cores: int = 8, T: int = N // 2, **spmd_kwargs):
    in_maps, flags = prep_inputs(inputs, n_cores, T)
    nc = _get_nc(n_cores, T, tuple(flags.values()))
    res = run_bass_kernel_spmd(nc, in_maps, core_ids=list(range(n_cores)),
                               **spmd_kwargs)
    out = np.zeros((B, N, C), np.float32)
    for c in range(n_cores):
        b, half = c // 2, c % 2
        out[b, half * T : (half + 1) * T, :] = np.asarray(
            res.results[c]["y"]).astype(np.float32)
    return out, res


def kernel(**inputs) -> np.ndarray:
    out, _ = run(inputs)
    return out
